# revision 9
# baseline (speedup 1.0000x reference)
"""Multi-head self-attention with additive position bias, data-parallel across
8 TRN2 NeuronCores (one batch element per core).

Per core (batch b), computed in a transposed layout so no on-device transposes
of the big GEMM operands are needed:
  - host supplies xT = x[b].T (fp16) and epos[h] = exp(pos[h].T / sqrt(D)) (fp16)
  - qT/kT    = W_{q,k}.T @ xT                    [cols, N]   (PE, fp16)
  - v        = xT.T @ W_v                        [N, cols]   (PE, fp16), with a
               literal ones-column appended after each head's 64 v-columns
  - scoresT  = kT_h(m-tile).T @ qT_h             [m, n]      (PE, head-pairs
               packed into row groups 0-63 / 64-127 of the systolic array)
  - estT     = exp(scoresT/sqrt(D)) * eposT      (ACT exp + DVE mul; the
               additive bias becomes a multiplicative factor after exp)
  - out_h    = estT.T @ v_aug,h : [n, 65] accumulated over m-tiles. Column 64
               (the ones-column) is the softmax denominator for row n — it
               lands in the SAME partition as the outputs it normalizes, so
               normalization is a reciprocal + per-partition scaled copy.
  - attnT    = PE-transpose of the normalized [n, 2 heads * 64] blocks
  - out      = attnT.T @ W_proj                  [N, C] fp32
"""

import numpy as np

N_CORES = 8
N = 1024
C = 768
H = 12
D = 64
HP = H // 2  # head pairs
SCALE = 0.125  # 1/sqrt(D)

# ---------------------------------------------------------------------------
# walrus in this toolchain rejects instructions carrying more than one sync
# wait ("Too many sync wait commands").  Tile's semaphore pass can attach
# several (esp. the kernel-tail drain).  Spread surplus waits across InstNoOp
# instructions inserted immediately before the oversubscribed instruction in
# the same basic block / engine stream — semantically identical, since the
# engine sequencer performs the waits in stream order.
# ---------------------------------------------------------------------------


def _apply_tile_patch():
    from concourse import mybir
    from concourse.tile import TileContext
    from concourse.vector_clock import ScopedClock

    def _patched_drain_and_barrier(self, tick_clock, wait_clock):
        nc = self.nc
        drain_inst = nc.sync.drain()
        wait_clock.add_sem_waits(
            drain_inst.ins, ScopedClock({None: tick_clock.global_clock})
        )
        nc.all_engine_barrier()
        assert self.sems is not None
        popped = nc._tile_sem_poison_stack.pop()
        assert popped is self._sem_poison
        nc.clear_and_free_semaphores(list(self.sems.allocated().values()))
        nc.all_engine_barrier()

    TileContext._drain_and_barrier = _patched_drain_and_barrier


def _split_excess_waits(nc, max_waits=1):
    from concourse import mybir

    n_split = 0
    for f in nc.m.functions:
        for blk in f.blocks:
            insts = blk.instructions
            new_list = []
            changed = False
            for inst in insts:
                si = inst.sync_info
                waits = list(si.on_wait) if (si is not None and si.on_wait) else []
                if len(waits) > max_waits:
                    extra = waits[: len(waits) - max_waits]
                    keep = waits[len(waits) - max_waits :]
                    for i in range(0, len(extra), max_waits):
                        nop = mybir.InstNoOp(
                            name=nc.get_next_instruction_name(),
                            engine=inst.engine,
                            ins=[],
                            outs=[],
                            sync_info=mybir.SyncInfo(
                                on_wait=extra[i : i + max_waits], on_update=[]
                            ),
                        )
                        nc.register_instruction(nop, overwrite=True)
                        new_list.append(nop)
                        n_split += 1
                    inst.sync_info = mybir.SyncInfo(
                        on_wait=keep,
                        on_update=list(si.on_update) if si.on_update else [],
                    )
                    changed = True
                new_list.append(inst)
            if changed:
                blk.instructions = new_list
    return n_split


def build(has_bias):
    import concourse.bass as bass
    import concourse.mybir as mybir
    from concourse import masks
    from concourse.tile import TileContext

    _apply_tile_patch()

    FP16 = mybir.dt.float16
    F32 = mybir.dt.float32
    EXP = mybir.ActivationFunctionType.Exp

    nc = bass.Bass()
    xt_ext = nc.declare_dram_parameter("xt", [C, N], FP16, isOutput=False)
    wqkv_ext = nc.declare_dram_parameter("wqkv", [C, 3 * C], FP16, isOutput=False)
    wproj_ext = nc.declare_dram_parameter("wproj", [C, C], FP16, isOutput=False)
    epos_ext = nc.declare_dram_parameter("epos", [H, N, N], FP16, isOutput=False)
    if has_bias:
        bqkv_ext = nc.declare_dram_parameter("bqkv", [1, 3 * C], FP16, isOutput=False)
        bproj_ext = nc.declare_dram_parameter("bproj", [1, C], FP16, isOutput=False)
    out_ext = nc.declare_dram_parameter("out", [N, C], F32, isOutput=True)

    KT = C // 128  # 6 contraction tiles
    NT = N // 128  # 8 n-tiles / m-tiles
    E = D + 1  # 65: v columns + ones column per head

    with TileContext(nc) as tc:
        with (
            tc.tile_pool(name="const", bufs=1) as const,
            tc.tile_pool(name="epp", bufs=6) as epp_pool,
            tc.tile_pool(name="est", bufs=16) as est_pool,
            tc.tile_pool(name="stg", bufs=3) as stg_pool,
            tc.tile_pool(name="rinv", bufs=3) as rinv_pool,
            tc.tile_pool(name="outsb", bufs=2) as outsb_pool,
            tc.tile_pool(name="ps", bufs=1, space="PSUM") as ps,
        ):
            XT = const.tile([128, KT, N], FP16)
            WQKV = const.tile([128, KT, 3 * C], FP16)
            WPROJ = const.tile([128, KT, C], FP16)
            xt_r = xt_ext.rearrange("(t p) n -> p t n", p=128)
            wqkv_r = wqkv_ext.rearrange("(t p) n -> p t n", p=128)
            # per-k-tile pieces, v columns first: the v-projection can start
            # as soon as the first k-tile of x and Wv has landed
            for kt in range(KT):
                nc.sync.dma_start(out=XT[:, kt, :], in_=xt_r[:, kt, :])
                nc.sync.dma_start(
                    out=WQKV[:, kt, 2 * C : 3 * C], in_=wqkv_r[:, kt, 2 * C : 3 * C]
                )
            nc.sync.dma_start(out=WQKV[:, :, 0 : 2 * C], in_=wqkv_r[:, :, 0 : 2 * C])
            nc.sync.dma_start(out=WPROJ[:], in_=wproj_ext.rearrange("(t p) n -> p t n", p=128))
            if has_bias:
                BQKV = const.tile([1, 3 * C], FP16)
                BPROJ = const.tile([1, C], FP16)
                ONESROW = const.tile([1, N], FP16)
                nc.sync.dma_start(out=BQKV[:], in_=bqkv_ext[:])
                nc.sync.dma_start(out=BPROJ[:], in_=bproj_ext[:])
                nc.vector.memset(ONESROW[:], 1.0)

            IDN = const.tile([128, 128], FP16)
            masks.make_identity(nc, IDN[:])

            # per pair hp: qT of heads (2hp, 2hp+1) at [0:N], kT at [N:2N]
            QKT = const.tile([128, HP, 2 * N], FP16)
            # v in [n, col] layout; after each head's 64 columns sits a
            # literal 1.0 column, so est.T @ VN65[head] yields the softmax
            # denominator in output column 64 — same partition as row n.
            VN65 = const.tile([128, NT, H * E], FP16)
            for h in range(H):
                nc.vector.memset(VN65[:, :, h * E + D : h * E + E], 1.0)
            ATTNT = const.tile([128, KT, N], FP16)

            # ---- V projection: v[n, vcol] = xT.T @ Wv (+ b_v) ----
            _vtags = ["sc", "out", "tp", "bc"]

            def _ps_tile(shape, tag, dtype=F32):
                return ps.tile(
                    shape, dtype, tag=tag, bufs=2 if tag == "out" else 1, name=f"ps_{tag}"
                )

            for nt in range(NT):
                for vs in range(2):
                    pv = _ps_tile([128, 384], _vtags[(nt * 2 + vs) % 4])
                    dst = pv[:, 0:384]
                    for kt in range(KT):
                        nc.tensor.matmul(
                            dst,
                            XT[:, kt, nt * 128 : (nt + 1) * 128],
                            WQKV[:, kt, 2 * C + vs * 384 : 2 * C + (vs + 1) * 384],
                            start=(kt == 0),
                            stop=(kt == KT - 1 and not has_bias),
                        )
                    if has_bias:
                        nc.tensor.matmul(
                            dst,
                            ONESROW[0:1, nt * 128 : (nt + 1) * 128],
                            BQKV[0:1, 2 * C + vs * 384 : 2 * C + (vs + 1) * 384],
                            start=False,
                            stop=True,
                        )
                    nc.vector.tensor_copy(
                        VN65[:, nt, vs * 6 * E : (vs + 1) * 6 * E].rearrange(
                            "p (h e) -> p h e", e=E
                        )[:, :, 0:D],
                        dst.rearrange("p (h d) -> p h d", d=D),
                    )

            # ---- head-pair loop, software-pipelined one pair deep:
            # pair hp:   scores -> exp -> est     (ACT-bound phase)
            # pair hp-1: est.T @ v_aug            (dense PE work, fills gaps)
            # pair hp+1: qT/kT projection chunks  (always-ready PE filler that
            #            keeps the HAM activity window busy -> PE stays warm)

            qkt_state = {}

            def qkt_half(pair, mt):
                # 24 qkT matmuls spread 4-per-mt over mts 0..5, so the final
                # cast lands two mts before the next pair's scores need it.
                # chunk c = (q ns0, q ns1, k ns0, k ns1); 6 matmuls per chunk.
                if mt >= 6:
                    return
                for j in range(4 * mt, 4 * mt + 4):
                    c, kt = j // 6, j % 6
                    ct = pair if c < 2 else HP + pair
                    col0 = ct * 128
                    ns = c % 2
                    if kt == 0:
                        pqc_t = _ps_tile([128, 512], "bc")
                        qkt_state[pair] = pqc_t
                    pqc = qkt_state[pair]
                    nc.tensor.matmul(
                        pqc[:],
                        WQKV[:, kt, col0 : col0 + 128],
                        XT[:, kt, ns * 512 : (ns + 1) * 512],
                        start=(kt == 0),
                        stop=(kt == KT - 1 and not has_bias),
                    )
                    if kt == KT - 1:
                        if has_bias:
                            nc.tensor.matmul(
                                pqc[:],
                                BQKV[0:1, col0 : col0 + 128],
                                ONESROW[0:1, ns * 512 : (ns + 1) * 512],
                                start=False,
                                stop=True,
                            )
                        nc.vector.tensor_copy(
                            QKT[:, pair, c * 512 : (c + 1) * 512], pqc[:]
                        )

            for mt in range(8):
                qkt_half(0, mt)

            def flush_block(ph, nt, OUT):
                # OUT [128, 2*E]: cols 0:64 = head0 out, 64 = head0 denom,
                # 65:129 = head1 out, 129 = head1 denom (all for rows n of
                # block nt).  Normalize per-partition, stage as fp16, and
                # leave the PE transpose + ATTNT copy to the caller.
                RINV = rinv_pool.tile([128, 2], F32, tag="rinv")
                nc.vector.reciprocal(
                    RINV[:].rearrange("p (h e) -> p h e", e=1),
                    OUT[:].rearrange("p (h e) -> p h e", e=E)[:, :, D : D + 1],
                )
                STG = stg_pool.tile([128, 128], FP16, tag="stg")
                nc.scalar.mul(STG[:, 0:D], OUT[:, 0:D], RINV[:, 0:1])
                nc.vector.tensor_scalar_mul(
                    STG[:, D : 2 * D], OUT[:, E : E + D], RINV[:, 1:2]
                )
                return STG

            prev = None  # (hp, [EST per mt])
            for hp in range(HP + 1):
                if hp < HP:
                    h0, h1 = 2 * hp, 2 * hp + 1
                cur = []
                pstate = None  # (nt, OUT) awaiting normalize
                tstate = None  # (nt, STG) awaiting transpose
                if prev is not None:
                    ph, pest = prev
                for mt in range(8):
                    # phase 2 of the previous pair first: its inputs are all
                    # ready, so the PE never stalls entering an iteration.
                    # Block nt=mt: out_h[n, 0:65] = sum_mt est_h.T @ v_aug,h
                    if prev is not None:
                        nt = mt
                        # Both heads' accumulation groups live in ONE psum
                        # bank (2KB zero region): the first matmul's start
                        # zeroes the whole region, so the h1 group simply
                        # accumulates into its zeroed columns; a single stop
                        # on the last matmul closes the region.
                        OUT = _ps_tile([128, 2 * E], "out")
                        for kt in range(8):
                            nsl = slice(nt * 128, (nt + 1) * 128)
                            nsl1 = slice(N + nt * 128, N + (nt + 1) * 128)
                            nc.tensor.matmul(
                                OUT[:, 0:E],
                                pest[kt][:, nsl],
                                VN65[:, kt, 2 * ph * E : (2 * ph + 1) * E],
                                start=(kt == 0), stop=False,
                            )
                            nc.tensor.matmul(
                                OUT[:, E : 2 * E],
                                pest[kt][:, nsl1],
                                VN65[:, kt, (2 * ph + 1) * E : (2 * ph + 2) * E],
                                start=False, stop=(kt == 7),
                            )
                        # normalize the previous block first (its OUT psum is
                        # complete; the DVE reciprocal has no PE dependency),
                        # then transpose the block normalized one iteration
                        # earlier (its STG is long ready -> no PE stall).
                        nstate = None
                        if pstate is not None:
                            pnt, POUT = pstate
                            nstate = (pnt, flush_block(ph, pnt, POUT))
                        if tstate is not None:
                            tnt, TSTG = tstate
                            TP = _ps_tile([128, 128], "tp", FP16)
                            nc.tensor.transpose(TP[:], TSTG[:], IDN[:])
                            nc.vector.tensor_copy(
                                ATTNT[:, ph, tnt * 128 : (tnt + 1) * 128], TP[:]
                            )
                        tstate = nstate
                        pstate = (nt, OUT)

                    # next pair's qkT half-chunk: elastic PE filler
                    if hp + 1 < HP:
                        qkt_half(hp + 1, mt)

                    # phase 1 of the current pair
                    if hp < HP:
                        EPP = epp_pool.tile([128, 2 * N], FP16, tag="epp")
                        nc.sync.dma_start(out=EPP[:, 0:N], in_=epos_ext[h0, mt * 128 : (mt + 1) * 128, :])
                        nc.sync.dma_start(out=EPP[:, N : 2 * N], in_=epos_ext[h1, mt * 128 : (mt + 1) * 128, :])

                        SCP = _ps_tile([128, 2 * N], "sc")
                        for ns in range(2):
                            nsl = slice(ns * 512, (ns + 1) * 512)
                            nsl1 = slice(N + ns * 512, N + (ns + 1) * 512)
                            nc.tensor.matmul(
                                SCP[:, nsl],
                                QKT[0:64, hp, N + mt * 128 : N + (mt + 1) * 128],
                                QKT[0:64, hp, nsl],
                                start=True, stop=True,
                            )
                            nc.tensor.matmul(
                                SCP[:, nsl1],
                                QKT[64:128, hp, N + mt * 128 : N + (mt + 1) * 128],
                                QKT[64:128, hp, nsl],
                                start=True, stop=True,
                            )

                        ESTP = est_pool.tile([128, 2 * N], FP16, tag="est")
                        nc.scalar.activation(ESTP[:], SCP[:], EXP, scale=SCALE)
                        nc.vector.tensor_mul(ESTP[:], ESTP[:], EPP[:])
                        cur.append(ESTP)

                if prev is not None:
                    # drain the last two blocks of pair ph
                    ph = prev[0]
                    for _ in range(2):
                        nstate = None
                        if pstate is not None:
                            pnt, POUT = pstate
                            nstate = (pnt, flush_block(ph, pnt, POUT))
                            pstate = None
                        if tstate is not None:
                            tnt, TSTG = tstate
                            TP = _ps_tile([128, 128], "tp", FP16)
                            nc.tensor.transpose(TP[:], TSTG[:], IDN[:])
                            nc.vector.tensor_copy(
                                ATTNT[:, ph, tnt * 128 : (tnt + 1) * 128], TP[:]
                            )
                        tstate = nstate

                if hp < HP:
                    prev = (hp, cur)

            # ---- output projection: out[n, c'] = attnT.T @ Wproj (+ b) ----
            for nt in range(NT):
                OF = outsb_pool.tile([128, C], F32, tag="of")
                for cs in range(2):
                    po = _ps_tile([128, 384], _vtags[(nt * 2 + cs) % 4])
                    dst = po[:, 0:384]
                    for ct in range(KT):
                        nc.tensor.matmul(
                            dst,
                            ATTNT[:, ct, nt * 128 : (nt + 1) * 128],
                            WPROJ[:, ct, cs * 384 : (cs + 1) * 384],
                            start=(ct == 0),
                            stop=(ct == KT - 1 and not has_bias),
                        )
                    if has_bias:
                        nc.tensor.matmul(
                            dst,
                            ONESROW[0:1, nt * 128 : (nt + 1) * 128],
                            BPROJ[0:1, cs * 384 : (cs + 1) * 384],
                            start=False,
                            stop=True,
                        )
                    nc.vector.tensor_copy(OF[:, cs * 384 : (cs + 1) * 384], dst)
                nc.sync.dma_start(out=out_ext[nt * 128 : (nt + 1) * 128, :], in_=OF[:])

    _split_excess_waits(nc)
    return nc


_BUILT = {}


def _get_nc(has_bias):
    if has_bias not in _BUILT:
        _BUILT[has_bias] = build(has_bias)
    return _BUILT[has_bias]


def prepare_inputs(x, pos_embedding, W_qkv, b_qkv, W_proj, b_proj):
    B = x.shape[0]
    has_bias = bool(np.any(b_qkv)) or bool(np.any(b_proj))
    wqkv16 = np.ascontiguousarray(W_qkv).astype(np.float16)
    wproj16 = np.ascontiguousarray(W_proj).astype(np.float16)
    epos16 = np.exp(
        pos_embedding[0].transpose(0, 2, 1).astype(np.float32) * SCALE
    ).astype(np.float16)
    epos16 = np.ascontiguousarray(epos16)
    in_maps = []
    for b in range(B):
        m = {
            "xt": np.ascontiguousarray(x[b].T).astype(np.float16),
            "wqkv": wqkv16,
            "wproj": wproj16,
            "epos": epos16,
        }
        if has_bias:
            m["bqkv"] = b_qkv.reshape(1, -1).astype(np.float16)
            m["bproj"] = b_proj.reshape(1, -1).astype(np.float16)
        in_maps.append(m)
    return has_bias, in_maps


def kernel(x, pos_embedding, W_qkv, b_qkv, W_proj, b_proj):
    from concourse.bass_utils import run_bass_kernel_spmd

    x = np.asarray(x)
    pos_embedding = np.asarray(pos_embedding)
    W_qkv = np.asarray(W_qkv)
    b_qkv = np.asarray(b_qkv)
    W_proj = np.asarray(W_proj)
    b_proj = np.asarray(b_proj)

    has_bias, in_maps = prepare_inputs(x, pos_embedding, W_qkv, b_qkv, W_proj, b_proj)
    nc = _get_nc(has_bias)
    res = run_bass_kernel_spmd(nc, in_maps, list(range(N_CORES)), trace=False)
    out = np.stack([res.results[i]["out"] for i in range(N_CORES)], axis=0)
    return out.astype(np.float32)


# revision 10
# speedup vs baseline: 1.0841x; 1.0841x over previous
"""Multi-head self-attention with additive position bias, data-parallel across
8 TRN2 NeuronCores (one batch element per core).

Per core (batch b), everything is computed in a transposed layout so that no
on-device transposes are needed:
  - host supplies xT = x[b].T (fp16) and epos[h] = exp(pos[h].T / sqrt(D)) (fp16)
  - qT/kT    = W_{q,k}.T @ xT                    [cols, N]   (PE, fp16)
  - v        = xT.T @ W_v                        [N, cols]   (PE, fp16), stored
               with a literal 1.0 column after each head's 64 columns
  - scoresT  = kT_h(m-tile).T @ qT_h             [m, n]      (PE, head-pairs
               packed into row groups 0-63 / 64-127 of the systolic array)
  - estT     = exp(scoresT/sqrt(D)) * eposT      (ACT exp + DVE mul; the
               additive bias becomes a multiplicative factor after exp)
  - outT_h   = v_aug,h.T @ estT : [65, n] accumulated over m-tiles.  The
               stationary operand is the 65-column v_aug (cheap LDWEIGHTS,
               fully hidden under the 512-element streams), and partition 64
               receives the softmax denominators for free — no separate
               ones-vector matmuls.
  - normalization: denominators are reshaped across lanes via two tiny DMAs,
    inverted on DVE, DMA'd back to [1, N] rows, broadcast with one-row
    matmuls, and multiplied into ATTNT.  The broadcast is deferred into the
    NEXT pair's loop so the small-DMA latency chain never stalls the PE.
  - out      = attnT.T @ W_proj                  [N, C] fp32
"""

import numpy as np

N_CORES = 8
N = 1024
C = 768
H = 12
D = 64
E = D + 1  # 65: v columns + ones column per head
HP = H // 2  # head pairs
SCALE = 0.125  # 1/sqrt(D)

# ---------------------------------------------------------------------------
# walrus in this toolchain rejects instructions carrying more than one sync
# wait ("Too many sync wait commands").  Tile's semaphore pass can attach
# several (esp. the kernel-tail drain).  Spread surplus waits across InstNoOp
# instructions inserted immediately before the oversubscribed instruction in
# the same basic block / engine stream — semantically identical, since the
# engine sequencer performs the waits in stream order.
# ---------------------------------------------------------------------------


def _apply_tile_patch():
    from concourse import mybir
    from concourse.tile import TileContext
    from concourse.vector_clock import ScopedClock

    def _patched_drain_and_barrier(self, tick_clock, wait_clock):
        nc = self.nc
        drain_inst = nc.sync.drain()
        wait_clock.add_sem_waits(
            drain_inst.ins, ScopedClock({None: tick_clock.global_clock})
        )
        nc.all_engine_barrier()
        assert self.sems is not None
        popped = nc._tile_sem_poison_stack.pop()
        assert popped is self._sem_poison
        nc.clear_and_free_semaphores(list(self.sems.allocated().values()))
        nc.all_engine_barrier()

    TileContext._drain_and_barrier = _patched_drain_and_barrier


def _split_excess_waits(nc, max_waits=1):
    from concourse import mybir

    n_split = 0
    for f in nc.m.functions:
        for blk in f.blocks:
            insts = blk.instructions
            new_list = []
            changed = False
            for inst in insts:
                si = inst.sync_info
                waits = list(si.on_wait) if (si is not None and si.on_wait) else []
                if len(waits) > max_waits:
                    extra = waits[: len(waits) - max_waits]
                    keep = waits[len(waits) - max_waits :]
                    for i in range(0, len(extra), max_waits):
                        nop = mybir.InstNoOp(
                            name=nc.get_next_instruction_name(),
                            engine=inst.engine,
                            ins=[],
                            outs=[],
                            sync_info=mybir.SyncInfo(
                                on_wait=extra[i : i + max_waits], on_update=[]
                            ),
                        )
                        nc.register_instruction(nop, overwrite=True)
                        new_list.append(nop)
                        n_split += 1
                    inst.sync_info = mybir.SyncInfo(
                        on_wait=keep,
                        on_update=list(si.on_update) if si.on_update else [],
                    )
                    changed = True
                new_list.append(inst)
            if changed:
                blk.instructions = new_list
    return n_split


def build(has_bias):
    import concourse.bass as bass
    import concourse.mybir as mybir
    from concourse.tile import TileContext

    _apply_tile_patch()

    FP16 = mybir.dt.float16
    F32 = mybir.dt.float32
    EXP = mybir.ActivationFunctionType.Exp

    nc = bass.Bass()
    xt_ext = nc.declare_dram_parameter("xt", [C, N], FP16, isOutput=False)
    wqkv_ext = nc.declare_dram_parameter("wqkv", [C, 3 * C], FP16, isOutput=False)
    wproj_ext = nc.declare_dram_parameter("wproj", [C, C], FP16, isOutput=False)
    epos_ext = nc.declare_dram_parameter("epos", [H, N, N], FP16, isOutput=False)
    if has_bias:
        bqkv_ext = nc.declare_dram_parameter("bqkv", [1, 3 * C], FP16, isOutput=False)
        bproj_ext = nc.declare_dram_parameter("bproj", [1, C], FP16, isOutput=False)
    out_ext = nc.declare_dram_parameter("out", [N, C], F32, isOutput=True)

    KT = C // 128  # 6 contraction tiles
    NT = N // 128  # 8 n-tiles / m-tiles

    with TileContext(nc) as tc:
        with (
            tc.tile_pool(name="const", bufs=1) as const,
            tc.tile_pool(name="epp", bufs=4) as epp_pool,
            tc.tile_pool(name="est", bufs=16) as est_pool,
            tc.tile_pool(name="qkt", bufs=3) as qkt_pool,
            tc.tile_pool(name="sgp", bufs=1) as sgp_pool,
            tc.tile_pool(name="stg", bufs=2) as stg_pool,
            tc.tile_pool(name="s2p", bufs=2) as s2_pool,
            tc.tile_pool(name="invr", bufs=4) as invr_pool,
            tc.tile_pool(name="outsb", bufs=2) as outsb_pool,
            tc.tile_pool(name="ps", bufs=1, space="PSUM") as ps,
        ):
            def _ps_tile(shape, tag):
                return ps.tile(
                    shape, F32, tag=tag, bufs=2 if tag == "bc" else 1,
                    name=f"ps_{tag}",
                )

            XT = const.tile([128, KT, N], FP16)
            WQKV = const.tile([128, KT, 3 * C], FP16)
            WPROJ = const.tile([128, KT, C], FP16)
            xt_r = xt_ext.rearrange("(t p) n -> p t n", p=128)
            wqkv_r = wqkv_ext.rearrange("(t p) n -> p t n", p=128)
            # per-k-tile pieces, v columns first: the v-projection can start
            # as soon as the first k-tile of x and Wv has landed
            for kt in range(KT):
                nc.sync.dma_start(out=XT[:, kt, :], in_=xt_r[:, kt, :])
                nc.sync.dma_start(
                    out=WQKV[:, kt, 2 * C : 3 * C], in_=wqkv_r[:, kt, 2 * C : 3 * C]
                )
            nc.sync.dma_start(out=WQKV[:, :, 0 : 2 * C], in_=wqkv_r[:, :, 0 : 2 * C])
            nc.sync.dma_start(out=WPROJ[:], in_=wproj_ext.rearrange("(t p) n -> p t n", p=128))
            if has_bias:
                BQKV = const.tile([1, 3 * C], FP16)
                BPROJ = const.tile([1, C], FP16)
                ONESROW = const.tile([1, N], FP16)
                nc.sync.dma_start(out=BQKV[:], in_=bqkv_ext[:])
                nc.sync.dma_start(out=BPROJ[:], in_=bproj_ext[:])
                nc.vector.memset(ONESROW[:], 1.0)

            ONES1x64 = const.tile([1, 64], FP16)
            nc.vector.memset(ONES1x64[:], 1.0)

            # v in [n, col] layout with a 1.0 column after each head's 64
            # columns: with v_aug as the matmul's stationary operand, output
            # partition 64 accumulates the softmax denominators for free.
            VN65 = const.tile([128, NT, H * E], FP16)
            for h in range(H):
                nc.vector.memset(VN65[:, :, h * E + D : h * E + E], 1.0)
            ATTNT = const.tile([128, KT, N], FP16)
            # softmax denominators parked at partition 64 (same partition the
            # attn@v matmuls write them to — engines cannot shift partitions)
            SGP = sgp_pool.tile([65, 2 * N], F32, tag="sgp")

            # ---- V projection: v[n, vcol] = xT.T @ Wv (+ b_v) ----
            _vtags = ["sc", "ot", "bc", "bc"]
            for nt in range(NT):
                for vs in range(2):
                    pv = _ps_tile([128, 384], _vtags[(nt * 2 + vs) % 4])
                    dst = pv[:, 0:384]
                    for kt in range(KT):
                        nc.tensor.matmul(
                            dst,
                            XT[:, kt, nt * 128 : (nt + 1) * 128],
                            WQKV[:, kt, 2 * C + vs * 384 : 2 * C + (vs + 1) * 384],
                            start=(kt == 0),
                            stop=(kt == KT - 1 and not has_bias),
                        )
                    if has_bias:
                        nc.tensor.matmul(
                            dst,
                            ONESROW[0:1, nt * 128 : (nt + 1) * 128],
                            BQKV[0:1, 2 * C + vs * 384 : 2 * C + (vs + 1) * 384],
                            start=False,
                            stop=True,
                        )
                    nc.vector.tensor_copy(
                        VN65[:, nt, vs * 6 * E : (vs + 1) * 6 * E].rearrange(
                            "p (h e) -> p h e", e=E
                        )[:, :, 0:D],
                        dst.rearrange("p (h d) -> p h d", d=D),
                    )

            # ---- head-pair loop, software-pipelined one pair deep:
            # pair hp:   scores -> exp -> est     (ACT-bound phase)
            # pair hp-1: v_aug.T @ est            (dense PE work, fills gaps)
            # pair hp+1: qT/kT projection chunks  (always-ready PE filler)
            # pair hp-2: deferred normalize broadcast (hides sums-DMA latency)

            qkt_state = {}
            qkt_tiles = {}

            def qkt_half(pair, mt):
                # 24 qkT matmuls spread 4-per-mt over mts 0..5, so the final
                # cast lands two mts before the next pair's scores need it.
                # chunk c = (q ns0, q ns1, k ns0, k ns1); 6 matmuls per chunk.
                if mt >= 6:
                    return
                if mt == 0:
                    qkt_tiles[pair] = qkt_pool.tile(
                        [128, 2 * N], FP16, tag="qkt", name=f"qkt_{pair}"
                    )
                for j in range(4 * mt, 4 * mt + 4):
                    c, kt = j // 6, j % 6
                    ct = pair if c < 2 else HP + pair
                    col0 = ct * 128
                    ns = c % 2
                    if kt == 0:
                        qkt_state[pair] = _ps_tile([128, 512], "bc")
                    pqc = qkt_state[pair]
                    nc.tensor.matmul(
                        pqc[:],
                        WQKV[:, kt, col0 : col0 + 128],
                        XT[:, kt, ns * 512 : (ns + 1) * 512],
                        start=(kt == 0),
                        stop=(kt == KT - 1 and not has_bias),
                    )
                    if kt == KT - 1:
                        if has_bias:
                            nc.tensor.matmul(
                                pqc[:],
                                BQKV[0:1, col0 : col0 + 128],
                                ONESROW[0:1, ns * 512 : (ns + 1) * 512],
                                start=False,
                                stop=True,
                            )
                        nc.vector.tensor_copy(
                            qkt_tiles[pair][:, c * 512 : (c + 1) * 512], pqc[:]
                        )

            for mt in range(8):
                qkt_half(0, mt)

            def do_norm(pend):
                # broadcast 1/sums rows to [128, 512] (head0 -> partitions
                # 0:63, head1 -> 64:127) and normalize ATTNT in place
                ph, IR0, IR1 = pend
                for ns in range(2):
                    nsl = slice(ns * 512, (ns + 1) * 512)
                    BCt = _ps_tile([128, 512], "bc")
                    nc.tensor.matmul(
                        BCt[0:64, :], ONES1x64[:], IR0[0:1, nsl],
                        start=True, stop=True,
                    )
                    nc.tensor.matmul(
                        BCt[64:128, :], ONES1x64[:], IR1[0:1, nsl],
                        start=True, stop=True,
                        tile_position=(0, 64),
                    )
                    nc.vector.tensor_mul(
                        ATTNT[:, ph, nsl], ATTNT[:, ph, nsl], BCt[:]
                    )

            norm_pending = None
            prev = None  # (hp, [EST per mt])
            OUTT = None
            for hp in range(HP + 1):
                if hp < HP:
                    h0, h1 = 2 * hp, 2 * hp + 1
                cur = []
                if prev is not None:
                    ph, pest = prev
                for mt in range(8):
                    # deferred normalize of pair hp-2: its 1/sums rows came
                    # back from the DMA shuffle during iterations 0-1
                    if norm_pending is not None and mt == 2:
                        do_norm(norm_pending)
                        norm_pending = None

                    # next pair's qkT half-chunk: elastic PE filler
                    if hp + 1 < HP:
                        qkt_half(hp + 1, mt)

                    # phase 1 of the current pair
                    if hp < HP:
                        EPP = epp_pool.tile([128, 2 * N], FP16, tag="epp")
                        nc.sync.dma_start(out=EPP[:, 0:N], in_=epos_ext[h0, mt * 128 : (mt + 1) * 128, :])
                        nc.sync.dma_start(out=EPP[:, N : 2 * N], in_=epos_ext[h1, mt * 128 : (mt + 1) * 128, :])

                        QKTh = qkt_tiles[hp]
                        SCP = _ps_tile([128, 2 * N], "sc")
                        for ns in range(2):
                            nsl = slice(ns * 512, (ns + 1) * 512)
                            nsl1 = slice(N + ns * 512, N + (ns + 1) * 512)
                            nc.tensor.matmul(
                                SCP[:, nsl],
                                QKTh[0:64, N + mt * 128 : N + (mt + 1) * 128],
                                QKTh[0:64, nsl],
                                start=True, stop=True,
                            )
                            nc.tensor.matmul(
                                SCP[:, nsl1],
                                QKTh[64:128, N + mt * 128 : N + (mt + 1) * 128],
                                QKTh[64:128, nsl],
                                start=True, stop=True,
                            )

                        ESTP = est_pool.tile([128, 2 * N], FP16, tag="est")
                        nc.scalar.activation(ESTP[:], SCP[:], EXP, scale=SCALE)
                        nc.vector.tensor_mul(ESTP[:], ESTP[:], EPP[:])
                        cur.append(ESTP)

                    # phase 2 of the previous pair (issued after the scores so
                    # an OUTT slot-reuse wait never blocks them): head0 of the
                    # pair sweeps est m-tiles 0..7 during mts 0-3, head1
                    # during mts 4-7.  outT_h[0:64] = attn@v rows, outT_h[64]
                    # = softmax denominators (from v_aug's ones column).
                    if prev is not None:
                        h_loc, sub = divmod(mt, 4)
                        hg = 2 * ph + h_loc
                        if sub == 0:
                            OUTT = _ps_tile([65, 1024], "ot")
                        for m2 in (2 * sub, 2 * sub + 1):
                            for ns in range(2):
                                nsl = slice(ns * 512, (ns + 1) * 512)
                                nc.tensor.matmul(
                                    OUTT[:, nsl],
                                    VN65[:, m2, hg * E : hg * E + E],
                                    pest[m2][:, h_loc * N + ns * 512 : h_loc * N + (ns + 1) * 512],
                                    start=(m2 == 0), stop=(m2 == 7),
                                )
                        if sub == 3:
                            # flush this head: rows 0:63 -> ATTNT (head0
                            # directly; head1 staged and repartitioned to
                            # 64:128 by an SBUF->SBUF DMA), denominator row
                            # 64 -> SGP (same partition).
                            nc.vector.tensor_copy(
                                SGP[64:65, h_loc * N : (h_loc + 1) * N],
                                OUTT[64:65, :],
                            )
                            if h_loc == 0:
                                nc.scalar.copy(ATTNT[0:64, ph, :], OUTT[0:64, :])
                            else:
                                STG1 = stg_pool.tile([64, N], FP16, tag="stg")
                                nc.scalar.copy(STG1[:], OUTT[0:64, :])
                                nc.sync.dma_start(
                                    out=ATTNT[64:128, ph, :], in_=STG1[:]
                                )
                                # reshape both denominator rows across 64
                                # lanes, invert, and send 1/sums back as
                                # [1, N] fp16 rows for the broadcast matmuls
                                S2 = s2_pool.tile([64, 32], F32, tag="s2")
                                for i in range(2):
                                    nc.sync.dma_start(
                                        out=S2[:, 16 * i : 16 * (i + 1)],
                                        in_=SGP[64:65, i * N : (i + 1) * N].rearrange(
                                            "o (p f) -> o p f", p=64
                                        ),
                                    )
                                RI = s2_pool.tile([64, 32], F32, tag="ri")
                                nc.vector.reciprocal(RI[:], S2[:])
                                RI16 = s2_pool.tile([64, 32], FP16, tag="ri16")
                                nc.vector.tensor_copy(RI16[:], RI[:])
                                IR0 = invr_pool.tile([1, N], FP16, tag="invr")
                                IR1 = invr_pool.tile([1, N], FP16, tag="invr")
                                for i, ir in enumerate((IR0, IR1)):
                                    nc.sync.dma_start(
                                        out=ir[0:1, :].rearrange(
                                            "o (p f) -> o p f", p=64
                                        ),
                                        in_=RI16[:, 16 * i : 16 * (i + 1)],
                                    )
                                norm_pending = (ph, IR0, IR1)

                if hp < HP:
                    prev = (hp, cur)

            # tail: normalize the last pair (the only place the sums-DMA
            # latency is exposed), then the output projection
            if norm_pending is not None:
                do_norm(norm_pending)
                norm_pending = None

            # ---- output projection: out[n, c'] = attnT.T @ Wproj (+ b) ----
            for nt in range(NT):
                OF = outsb_pool.tile([128, C], F32, tag="of")
                for cs in range(2):
                    po = _ps_tile([128, 384], _vtags[(nt * 2 + cs) % 4])
                    dst = po[:, 0:384]
                    for ct in range(KT):
                        nc.tensor.matmul(
                            dst,
                            ATTNT[:, ct, nt * 128 : (nt + 1) * 128],
                            WPROJ[:, ct, cs * 384 : (cs + 1) * 384],
                            start=(ct == 0),
                            stop=(ct == KT - 1 and not has_bias),
                        )
                    if has_bias:
                        nc.tensor.matmul(
                            dst,
                            ONESROW[0:1, nt * 128 : (nt + 1) * 128],
                            BPROJ[0:1, cs * 384 : (cs + 1) * 384],
                            start=False,
                            stop=True,
                        )
                    nc.vector.tensor_copy(OF[:, cs * 384 : (cs + 1) * 384], dst)
                nc.sync.dma_start(out=out_ext[nt * 128 : (nt + 1) * 128, :], in_=OF[:])

    _split_excess_waits(nc)
    return nc


_BUILT = {}


def _get_nc(has_bias):
    if has_bias not in _BUILT:
        _BUILT[has_bias] = build(has_bias)
    return _BUILT[has_bias]


def prepare_inputs(x, pos_embedding, W_qkv, b_qkv, W_proj, b_proj):
    B = x.shape[0]
    has_bias = bool(np.any(b_qkv)) or bool(np.any(b_proj))
    wqkv16 = np.ascontiguousarray(W_qkv).astype(np.float16)
    wproj16 = np.ascontiguousarray(W_proj).astype(np.float16)
    epos16 = np.exp(
        pos_embedding[0].transpose(0, 2, 1).astype(np.float32) * SCALE
    ).astype(np.float16)
    epos16 = np.ascontiguousarray(epos16)
    in_maps = []
    for b in range(B):
        m = {
            "xt": np.ascontiguousarray(x[b].T).astype(np.float16),
            "wqkv": wqkv16,
            "wproj": wproj16,
            "epos": epos16,
        }
        if has_bias:
            m["bqkv"] = b_qkv.reshape(1, -1).astype(np.float16)
            m["bproj"] = b_proj.reshape(1, -1).astype(np.float16)
        in_maps.append(m)
    return has_bias, in_maps


def kernel(x, pos_embedding, W_qkv, b_qkv, W_proj, b_proj):
    from concourse.bass_utils import run_bass_kernel_spmd

    x = np.asarray(x)
    pos_embedding = np.asarray(pos_embedding)
    W_qkv = np.asarray(W_qkv)
    b_qkv = np.asarray(b_qkv)
    W_proj = np.asarray(W_proj)
    b_proj = np.asarray(b_proj)

    has_bias, in_maps = prepare_inputs(x, pos_embedding, W_qkv, b_qkv, W_proj, b_proj)
    nc = _get_nc(has_bias)
    res = run_bass_kernel_spmd(nc, in_maps, list(range(N_CORES)), trace=False)
    out = np.stack([res.results[i]["out"] for i in range(N_CORES)], axis=0)
    return out.astype(np.float32)


# revision 13
# speedup vs baseline: 1.1317x; 1.0439x over previous
"""Multi-head self-attention with additive position bias, data-parallel across
8 TRN2 NeuronCores (one batch element per core).

Per core (batch b), everything is computed in a transposed layout so that no
on-device transposes are needed:
  - host supplies xT = x[b].T (fp16) and epos[h] = exp(pos[h].T / sqrt(D)) (fp16)
  - qT/kT    = W_{q,k}.T @ xT                    [cols, N]   (PE, fp16)
  - v        = xT.T @ W_v                        [N, cols]   (PE, fp16), stored
               with a literal 1.0 column after each head's 64 columns
  - scoresT  = kT_h(m-tile).T @ qT_h             [m, n]      (PE, head-pairs
               packed into row groups 0-63 / 64-127 of the systolic array)
  - estT     = exp(scoresT/sqrt(D)) * eposT      (ACT exp + DVE mul; the
               additive bias becomes a multiplicative factor after exp)
  - outT_h   = v_aug,h.T @ estT : [65, n] accumulated over m-tiles.  The
               stationary operand is the 65-column v_aug (cheap LDWEIGHTS,
               fully hidden under the 512-element streams), and partition 64
               receives the softmax denominators for free — no separate
               ones-vector matmuls.
  - normalization: denominators are reshaped across lanes via two tiny DMAs,
    inverted on DVE, DMA'd back to [1, N] rows, broadcast with one-row
    matmuls, and multiplied into ATTNT.  The broadcast is deferred into the
    NEXT pair's loop so the small-DMA latency chain never stalls the PE.
  - out      = attnT.T @ W_proj                  [N, C] fp32
"""

import numpy as np

N_CORES = 8
N = 1024
C = 768
H = 12
D = 64
E = D + 1  # 65: v columns + ones column per head
HP = H // 2  # head pairs
SCALE = 0.125  # 1/sqrt(D)

# ---------------------------------------------------------------------------
# walrus in this toolchain rejects instructions carrying more than one sync
# wait ("Too many sync wait commands").  Tile's semaphore pass can attach
# several (esp. the kernel-tail drain).  Spread surplus waits across InstNoOp
# instructions inserted immediately before the oversubscribed instruction in
# the same basic block / engine stream — semantically identical, since the
# engine sequencer performs the waits in stream order.
# ---------------------------------------------------------------------------


def _apply_tile_patch():
    from concourse import mybir
    from concourse.tile import TileContext
    from concourse.vector_clock import ScopedClock

    def _patched_drain_and_barrier(self, tick_clock, wait_clock):
        nc = self.nc
        drain_inst = nc.sync.drain()
        wait_clock.add_sem_waits(
            drain_inst.ins, ScopedClock({None: tick_clock.global_clock})
        )
        nc.all_engine_barrier()
        assert self.sems is not None
        popped = nc._tile_sem_poison_stack.pop()
        assert popped is self._sem_poison
        nc.clear_and_free_semaphores(list(self.sems.allocated().values()))
        nc.all_engine_barrier()

    TileContext._drain_and_barrier = _patched_drain_and_barrier


def _split_excess_waits(nc, max_waits=1):
    from concourse import mybir

    n_split = 0
    for f in nc.m.functions:
        for blk in f.blocks:
            insts = blk.instructions
            new_list = []
            changed = False
            for inst in insts:
                si = inst.sync_info
                waits = list(si.on_wait) if (si is not None and si.on_wait) else []
                if len(waits) > max_waits:
                    extra = waits[: len(waits) - max_waits]
                    keep = waits[len(waits) - max_waits :]
                    for i in range(0, len(extra), max_waits):
                        nop = mybir.InstNoOp(
                            name=nc.get_next_instruction_name(),
                            engine=inst.engine,
                            ins=[],
                            outs=[],
                            sync_info=mybir.SyncInfo(
                                on_wait=extra[i : i + max_waits], on_update=[]
                            ),
                        )
                        nc.register_instruction(nop, overwrite=True)
                        new_list.append(nop)
                        n_split += 1
                    inst.sync_info = mybir.SyncInfo(
                        on_wait=keep,
                        on_update=list(si.on_update) if si.on_update else [],
                    )
                    changed = True
                new_list.append(inst)
            if changed:
                blk.instructions = new_list
    return n_split


def build(has_bias):
    import concourse.bass as bass
    import concourse.mybir as mybir
    from concourse.tile import TileContext

    _apply_tile_patch()

    FP16 = mybir.dt.float16
    F32 = mybir.dt.float32
    EXP = mybir.ActivationFunctionType.Exp

    nc = bass.Bass()
    xt_ext = nc.declare_dram_parameter("xt", [C, N], FP16, isOutput=False)
    wqkv_ext = nc.declare_dram_parameter("wqkv", [C, 3 * C], FP16, isOutput=False)
    wproj_ext = nc.declare_dram_parameter("wproj", [C, C], FP16, isOutput=False)
    epos_ext = nc.declare_dram_parameter("epos", [H, N, N], FP16, isOutput=False)
    if has_bias:
        bqkv_ext = nc.declare_dram_parameter("bqkv", [1, 3 * C], FP16, isOutput=False)
        bproj_ext = nc.declare_dram_parameter("bproj", [1, C], FP16, isOutput=False)
    out_ext = nc.declare_dram_parameter("out", [N, C], F32, isOutput=True)

    KT = C // 128  # 6 contraction tiles
    NT = N // 128  # 8 n-tiles / m-tiles

    with TileContext(nc) as tc:
        with (
            tc.tile_pool(name="const", bufs=1) as const,
            tc.tile_pool(name="epp", bufs=4) as epp_pool,
            tc.tile_pool(name="est", bufs=16) as est_pool,
            tc.tile_pool(name="qkt", bufs=3) as qkt_pool,
            tc.tile_pool(name="sgp", bufs=1) as sgp_pool,
            tc.tile_pool(name="stg", bufs=2) as stg_pool,
            tc.tile_pool(name="s2p", bufs=2) as s2_pool,
            tc.tile_pool(name="invr", bufs=4) as invr_pool,
            tc.tile_pool(name="outsb", bufs=2) as outsb_pool,
            tc.tile_pool(name="ps", bufs=1, space="PSUM") as ps,
        ):
            def _ps_tile(shape, tag):
                return ps.tile(
                    shape, F32, tag=tag, bufs=2 if tag == "bc" else 1,
                    name=f"ps_{tag}",
                )

            XT = const.tile([128, KT, N], FP16)
            WQKV = const.tile([128, KT, 3 * C], FP16)
            WPROJ = const.tile([128, KT, C], FP16)
            xt_r = xt_ext.rearrange("(t p) n -> p t n", p=128)
            wqkv_r = wqkv_ext.rearrange("(t p) n -> p t n", p=128)
            # per-k-tile pieces, v columns first: the v-projection can start
            # as soon as the first k-tile of x and Wv has landed
            for kt in range(KT):
                nc.sync.dma_start(out=XT[:, kt, :], in_=xt_r[:, kt, :])
                nc.sync.dma_start(
                    out=WQKV[:, kt, 2 * C : 3 * C], in_=wqkv_r[:, kt, 2 * C : 3 * C]
                )
            nc.sync.dma_start(out=WQKV[:, :, 0 : 2 * C], in_=wqkv_r[:, :, 0 : 2 * C])
            nc.sync.dma_start(out=WPROJ[:], in_=wproj_ext.rearrange("(t p) n -> p t n", p=128))
            if has_bias:
                BQKV = const.tile([1, 3 * C], FP16)
                BPROJ = const.tile([1, C], FP16)
                ONESROW = const.tile([1, N], FP16)
                nc.sync.dma_start(out=BQKV[:], in_=bqkv_ext[:])
                nc.sync.dma_start(out=BPROJ[:], in_=bproj_ext[:])
                nc.vector.memset(ONESROW[:], 1.0)

            ONES1x64 = const.tile([1, 64], FP16)
            nc.vector.memset(ONES1x64[:], 1.0)

            # v in [n, col] layout with a 1.0 column after each head's 64
            # columns: with v_aug as the matmul's stationary operand, output
            # partition 64 accumulates the softmax denominators for free.
            VN65 = const.tile([128, NT, H * E], FP16)
            for h in range(H):
                nc.vector.memset(VN65[:, :, h * E + D : h * E + E], 1.0)
            ATTNT = const.tile([128, KT, N], FP16)
            # softmax denominators parked at partition 64 (same partition the
            # attn@v matmuls write them to — engines cannot shift partitions)
            SGP = sgp_pool.tile([65, 2 * N], F32, tag="sgp")

            # ---- V projection: v[n, vcol] = xT.T @ Wv (+ b_v) ----
            _vtags = ["sc", "ot", "bc", "bc"]
            for nt in range(NT):
                for vs in range(2):
                    pv = _ps_tile([128, 384], _vtags[(nt * 2 + vs) % 4])
                    dst = pv[:, 0:384]
                    for kt in range(KT):
                        nc.tensor.matmul(
                            dst,
                            XT[:, kt, nt * 128 : (nt + 1) * 128],
                            WQKV[:, kt, 2 * C + vs * 384 : 2 * C + (vs + 1) * 384],
                            start=(kt == 0),
                            stop=(kt == KT - 1 and not has_bias),
                        )
                    if has_bias:
                        nc.tensor.matmul(
                            dst,
                            ONESROW[0:1, nt * 128 : (nt + 1) * 128],
                            BQKV[0:1, 2 * C + vs * 384 : 2 * C + (vs + 1) * 384],
                            start=False,
                            stop=True,
                        )
                    nc.vector.tensor_copy(
                        VN65[:, nt, vs * 6 * E : (vs + 1) * 6 * E].rearrange(
                            "p (h e) -> p h e", e=E
                        )[:, :, 0:D],
                        dst.rearrange("p (h d) -> p h d", d=D),
                    )

            # ---- head-pair loop, software-pipelined one pair deep:
            # pair hp:   scores -> exp -> est     (ACT-bound phase)
            # pair hp-1: v_aug.T @ est            (dense PE work, fills gaps)
            # pair hp+1: qT/kT projection chunks  (always-ready PE filler)
            # pair hp-2: deferred normalize broadcast (hides sums-DMA latency)

            qkt_state = {}
            qkt_tiles = {}

            def qkt_half(pair, mt):
                # 24 qkT matmuls spread 4-per-mt over mts 0..5, so the final
                # cast lands two mts before the next pair's scores need it.
                # chunk c = (q ns0, q ns1, k ns0, k ns1); 6 matmuls per chunk.
                if mt >= 6:
                    return
                if mt == 0:
                    qkt_tiles[pair] = qkt_pool.tile(
                        [128, 2 * N], FP16, tag="qkt", name=f"qkt_{pair}"
                    )
                for j in range(4 * mt, 4 * mt + 4):
                    c, kt = j // 6, j % 6
                    ct = pair if c < 2 else HP + pair
                    col0 = ct * 128
                    ns = c % 2
                    if kt == 0:
                        qkt_state[pair] = _ps_tile([128, 512], "bc")
                    pqc = qkt_state[pair]
                    nc.tensor.matmul(
                        pqc[:],
                        WQKV[:, kt, col0 : col0 + 128],
                        XT[:, kt, ns * 512 : (ns + 1) * 512],
                        start=(kt == 0),
                        stop=(kt == KT - 1 and not has_bias),
                    )
                    if kt == KT - 1:
                        if has_bias:
                            nc.tensor.matmul(
                                pqc[:],
                                BQKV[0:1, col0 : col0 + 128],
                                ONESROW[0:1, ns * 512 : (ns + 1) * 512],
                                start=False,
                                stop=True,
                            )
                        nc.vector.tensor_copy(
                            qkt_tiles[pair][:, c * 512 : (c + 1) * 512], pqc[:]
                        )

            for mt in range(8):
                qkt_half(0, mt)

            def do_norm(pend):
                # broadcast 1/sums rows to [128, 512] (head0 -> partitions
                # 0:63, head1 -> 64:127) and normalize ATTNT in place
                ph, IR0, IR1 = pend
                for ns in range(2):
                    nsl = slice(ns * 512, (ns + 1) * 512)
                    BCt = _ps_tile([128, 512], "bc")
                    nc.tensor.matmul(
                        BCt[0:64, :], ONES1x64[:], IR0[0:1, nsl],
                        start=True, stop=True,
                    )
                    nc.tensor.matmul(
                        BCt[64:128, :], ONES1x64[:], IR1[0:1, nsl],
                        start=True, stop=True,
                        tile_position=(0, 64),
                    )
                    nc.vector.tensor_mul(
                        ATTNT[:, ph, nsl], ATTNT[:, ph, nsl], BCt[:]
                    )

            norm_pending = None
            prev = None  # (hp, [EST per mt])
            OUTT = None
            for hp in range(HP + 1):
                if hp < HP:
                    h0, h1 = 2 * hp, 2 * hp + 1
                cur = []
                if prev is not None:
                    ph, pest = prev
                for mt in range(8):
                    # deferred normalize of pair hp-2: its 1/sums rows came
                    # back from the DMA shuffle during iterations 0-1
                    if norm_pending is not None and mt == 2:
                        do_norm(norm_pending)
                        norm_pending = None

                    # next pair's qkT half-chunk: elastic PE filler
                    if hp + 1 < HP:
                        qkt_half(hp + 1, mt)

                    # phase 1 of the current pair (the est = exp * epos DVE
                    # multiply is deferred to the end of the iteration: its
                    # consumer is a whole pair-loop away, and issuing it last
                    # keeps the DVE free for the phase-2 flush copies that
                    # gate the next attn@v accumulation)
                    EPP = ESTP = None
                    if hp < HP:
                        EPP = epp_pool.tile([128, 2 * N], FP16, tag="epp")
                        nc.sync.dma_start(out=EPP[:, 0:N], in_=epos_ext[h0, mt * 128 : (mt + 1) * 128, :])
                        nc.sync.dma_start(out=EPP[:, N : 2 * N], in_=epos_ext[h1, mt * 128 : (mt + 1) * 128, :])

                        QKTh = qkt_tiles[hp]
                        SCP = _ps_tile([128, 2 * N], "sc")
                        for ns in range(2):
                            nsl = slice(ns * 512, (ns + 1) * 512)
                            nsl1 = slice(N + ns * 512, N + (ns + 1) * 512)
                            nc.tensor.matmul(
                                SCP[:, nsl],
                                QKTh[0:64, N + mt * 128 : N + (mt + 1) * 128],
                                QKTh[0:64, nsl],
                                start=True, stop=True,
                            )
                            nc.tensor.matmul(
                                SCP[:, nsl1],
                                QKTh[64:128, N + mt * 128 : N + (mt + 1) * 128],
                                QKTh[64:128, nsl],
                                start=True, stop=True,
                            )

                        ESTP = est_pool.tile([128, 2 * N], FP16, tag="est")
                        nc.scalar.activation(ESTP[:], SCP[:], EXP, scale=SCALE)
                        cur.append(ESTP)

                    # phase 2 of the previous pair (issued after the scores so
                    # an OUTT slot-reuse wait never blocks them): head0 of the
                    # pair sweeps est m-tiles 0..7 during mts 0-3, head1
                    # during mts 4-7.  outT_h[0:64] = attn@v rows, outT_h[64]
                    # = softmax denominators (from v_aug's ones column).
                    if prev is not None:
                        h_loc, sub = divmod(mt, 4)
                        hg = 2 * ph + h_loc
                        if sub == 0:
                            OUTT = _ps_tile([65, 1024], "ot")
                        for m2 in (2 * sub, 2 * sub + 1):
                            for ns in range(2):
                                nsl = slice(ns * 512, (ns + 1) * 512)
                                nc.tensor.matmul(
                                    OUTT[:, nsl],
                                    VN65[:, m2, hg * E : hg * E + E],
                                    pest[m2][:, h_loc * N + ns * 512 : h_loc * N + (ns + 1) * 512],
                                    start=(m2 == 0), stop=(m2 == 7),
                                )
                        if sub == 3:
                            # flush this head on DVE (ACT is saturated by the
                            # exp chain; GPSIMD has no PSUM port): rows 0:63
                            # -> ATTNT (head0 directly; head1 staged and
                            # repartitioned to 64:128 by an SBUF->SBUF DMA),
                            # denominator row 64 -> SGP (same partition).
                            nc.vector.tensor_copy(
                                SGP[64:65, h_loc * N : (h_loc + 1) * N],
                                OUTT[64:65, :],
                            )
                            if h_loc == 0:
                                nc.vector.tensor_copy(ATTNT[0:64, ph, :], OUTT[0:64, :])
                            else:
                                STG1 = stg_pool.tile([64, N], FP16, tag="stg")
                                nc.vector.tensor_copy(STG1[:], OUTT[0:64, :])
                                nc.sync.dma_start(
                                    out=ATTNT[64:128, ph, :], in_=STG1[:]
                                )
                                # reshape both denominator rows across 64
                                # lanes, invert, and send 1/sums back as
                                # [1, N] fp16 rows for the broadcast matmuls
                                S2 = s2_pool.tile([64, 32], F32, tag="s2")
                                for i in range(2):
                                    nc.sync.dma_start(
                                        out=S2[:, 16 * i : 16 * (i + 1)],
                                        in_=SGP[64:65, i * N : (i + 1) * N].rearrange(
                                            "o (p f) -> o p f", p=64
                                        ),
                                    )
                                RI = s2_pool.tile([64, 32], F32, tag="ri")
                                nc.vector.reciprocal(RI[:], S2[:])
                                RI16 = s2_pool.tile([64, 32], FP16, tag="ri16")
                                nc.vector.tensor_copy(RI16[:], RI[:])
                                IR0 = invr_pool.tile([1, N], FP16, tag="invr")
                                IR1 = invr_pool.tile([1, N], FP16, tag="invr")
                                for i, ir in enumerate((IR0, IR1)):
                                    nc.sync.dma_start(
                                        out=ir[0:1, :].rearrange(
                                            "o (p f) -> o p f", p=64
                                        ),
                                        in_=RI16[:, 16 * i : 16 * (i + 1)],
                                    )
                                norm_pending = (ph, IR0, IR1)

                    # deferred phase-1 tail: est = exp(scores) * epos
                    if ESTP is not None:
                        nc.vector.tensor_mul(ESTP[:], ESTP[:], EPP[:])

                if hp < HP:
                    prev = (hp, cur)

            # tail: normalize the last pair (the only place the sums-DMA
            # latency is exposed), then the output projection
            if norm_pending is not None:
                do_norm(norm_pending)
                norm_pending = None

            # ---- output projection: out[n, c'] = attnT.T @ Wproj (+ b) ----
            for nt in range(NT):
                OF = outsb_pool.tile([128, C], F32, tag="of")
                for cs in range(2):
                    po = _ps_tile([128, 384], _vtags[(nt * 2 + cs) % 4])
                    dst = po[:, 0:384]
                    for ct in range(KT):
                        nc.tensor.matmul(
                            dst,
                            ATTNT[:, ct, nt * 128 : (nt + 1) * 128],
                            WPROJ[:, ct, cs * 384 : (cs + 1) * 384],
                            start=(ct == 0),
                            stop=(ct == KT - 1 and not has_bias),
                        )
                    if has_bias:
                        nc.tensor.matmul(
                            dst,
                            ONESROW[0:1, nt * 128 : (nt + 1) * 128],
                            BPROJ[0:1, cs * 384 : (cs + 1) * 384],
                            start=False,
                            stop=True,
                        )
                    nc.vector.tensor_copy(OF[:, cs * 384 : (cs + 1) * 384], dst)
                nc.sync.dma_start(out=out_ext[nt * 128 : (nt + 1) * 128, :], in_=OF[:])

    _split_excess_waits(nc)
    return nc


_BUILT = {}


def _get_nc(has_bias):
    if has_bias not in _BUILT:
        _BUILT[has_bias] = build(has_bias)
    return _BUILT[has_bias]


def prepare_inputs(x, pos_embedding, W_qkv, b_qkv, W_proj, b_proj):
    B = x.shape[0]
    has_bias = bool(np.any(b_qkv)) or bool(np.any(b_proj))
    wqkv16 = np.ascontiguousarray(W_qkv).astype(np.float16)
    wproj16 = np.ascontiguousarray(W_proj).astype(np.float16)
    epos16 = np.exp(
        pos_embedding[0].transpose(0, 2, 1).astype(np.float32) * SCALE
    ).astype(np.float16)
    epos16 = np.ascontiguousarray(epos16)
    in_maps = []
    for b in range(B):
        m = {
            "xt": np.ascontiguousarray(x[b].T).astype(np.float16),
            "wqkv": wqkv16,
            "wproj": wproj16,
            "epos": epos16,
        }
        if has_bias:
            m["bqkv"] = b_qkv.reshape(1, -1).astype(np.float16)
            m["bproj"] = b_proj.reshape(1, -1).astype(np.float16)
        in_maps.append(m)
    return has_bias, in_maps


def kernel(x, pos_embedding, W_qkv, b_qkv, W_proj, b_proj):
    from concourse.bass_utils import run_bass_kernel_spmd

    x = np.asarray(x)
    pos_embedding = np.asarray(pos_embedding)
    W_qkv = np.asarray(W_qkv)
    b_qkv = np.asarray(b_qkv)
    W_proj = np.asarray(W_proj)
    b_proj = np.asarray(b_proj)

    has_bias, in_maps = prepare_inputs(x, pos_embedding, W_qkv, b_qkv, W_proj, b_proj)
    nc = _get_nc(has_bias)
    res = run_bass_kernel_spmd(nc, in_maps, list(range(N_CORES)), trace=False)
    out = np.stack([res.results[i]["out"] for i in range(N_CORES)], axis=0)
    return out.astype(np.float32)


# revision 17
# speedup vs baseline: 1.1922x; 1.0534x over previous
"""Multi-head self-attention with additive position bias, data-parallel across
8 TRN2 NeuronCores (one batch element per core).

Per core (batch b), everything is computed in a transposed layout so that no
on-device transposes are needed:
  - host supplies xT = x[b].T (fp16) and epos[h] = exp(pos[h].T / sqrt(D)) (fp16)
  - qT/kT    = W_{q,k}.T @ xT                    [cols, N]   (PE, fp16)
  - v        = xT.T @ W_v                        [N, cols]   (PE, fp16), stored
               with a literal 1.0 column after each head's 64 columns
  - scoresT  = kT_h(m-tile).T @ qT_h             [m, n]      (PE, head-pairs
               packed into row groups 0-63 / 64-127 of the systolic array)
  - estT     = exp(scoresT/sqrt(D)) * eposT      (ACT exp + DVE mul; the
               additive bias becomes a multiplicative factor after exp)
  - outT_h   = v_aug,h.T @ estT : [65, n] accumulated over m-tiles.  The
               stationary operand is the 65-column v_aug (cheap LDWEIGHTS,
               fully hidden under the 512-element streams), and partition 64
               receives the softmax denominators for free — no separate
               ones-vector matmuls.
  - normalization: denominators are reshaped across lanes via two tiny DMAs,
    inverted on DVE, DMA'd back to [1, N] rows, broadcast with one-row
    matmuls, and multiplied into ATTNT.  The broadcast is deferred into the
    NEXT pair's loop so the small-DMA latency chain never stalls the PE.
  - out      = attnT.T @ W_proj                  [N, C] fp32
"""

import numpy as np

N_CORES = 8
N = 1024
C = 768
H = 12
D = 64
E = D + 1  # 65: v columns + ones column per head
HP = H // 2  # head pairs
SCALE = 0.125  # 1/sqrt(D)

# ---------------------------------------------------------------------------
# walrus in this toolchain rejects instructions carrying more than one sync
# wait ("Too many sync wait commands").  Tile's semaphore pass can attach
# several (esp. the kernel-tail drain).  Spread surplus waits across InstNoOp
# instructions inserted immediately before the oversubscribed instruction in
# the same basic block / engine stream — semantically identical, since the
# engine sequencer performs the waits in stream order.
# ---------------------------------------------------------------------------


def _apply_tile_patch():
    from concourse import mybir
    from concourse.tile import TileContext
    from concourse.vector_clock import ScopedClock

    def _patched_drain_and_barrier(self, tick_clock, wait_clock):
        nc = self.nc
        drain_inst = nc.sync.drain()
        wait_clock.add_sem_waits(
            drain_inst.ins, ScopedClock({None: tick_clock.global_clock})
        )
        nc.all_engine_barrier()
        assert self.sems is not None
        popped = nc._tile_sem_poison_stack.pop()
        assert popped is self._sem_poison
        nc.clear_and_free_semaphores(list(self.sems.allocated().values()))
        nc.all_engine_barrier()

    TileContext._drain_and_barrier = _patched_drain_and_barrier


def _split_excess_waits(nc, max_waits=1):
    from concourse import mybir

    n_split = 0
    for f in nc.m.functions:
        for blk in f.blocks:
            insts = blk.instructions
            new_list = []
            changed = False
            for inst in insts:
                si = inst.sync_info
                waits = list(si.on_wait) if (si is not None and si.on_wait) else []
                if len(waits) > max_waits:
                    extra = waits[: len(waits) - max_waits]
                    keep = waits[len(waits) - max_waits :]
                    for i in range(0, len(extra), max_waits):
                        nop = mybir.InstNoOp(
                            name=nc.get_next_instruction_name(),
                            engine=inst.engine,
                            ins=[],
                            outs=[],
                            sync_info=mybir.SyncInfo(
                                on_wait=extra[i : i + max_waits], on_update=[]
                            ),
                        )
                        nc.register_instruction(nop, overwrite=True)
                        new_list.append(nop)
                        n_split += 1
                    inst.sync_info = mybir.SyncInfo(
                        on_wait=keep,
                        on_update=list(si.on_update) if si.on_update else [],
                    )
                    changed = True
                new_list.append(inst)
            if changed:
                blk.instructions = new_list
    return n_split


def build(has_bias):
    import concourse.bass as bass
    import concourse.mybir as mybir
    from concourse.tile import TileContext

    _apply_tile_patch()

    FP16 = mybir.dt.float16
    F32 = mybir.dt.float32
    EXP = mybir.ActivationFunctionType.Exp

    nc = bass.Bass()
    xt_ext = nc.declare_dram_parameter("xt", [C, N], FP16, isOutput=False)
    wqkv_ext = nc.declare_dram_parameter("wqkv", [C, 3 * C], FP16, isOutput=False)
    wproj_ext = nc.declare_dram_parameter("wproj", [C, C], FP16, isOutput=False)
    epos_ext = nc.declare_dram_parameter("epos", [H, N, N], FP16, isOutput=False)
    if has_bias:
        bqkv_ext = nc.declare_dram_parameter("bqkv", [1, 3 * C], FP16, isOutput=False)
        bproj_ext = nc.declare_dram_parameter("bproj", [1, C], FP16, isOutput=False)
    out_ext = nc.declare_dram_parameter("out", [N, C], F32, isOutput=True)

    KT = C // 128  # 6 contraction tiles
    NT = N // 128  # 8 n-tiles / m-tiles

    with TileContext(nc) as tc:
        with (
            tc.tile_pool(name="const", bufs=1) as const,
            tc.tile_pool(name="epp", bufs=4) as epp_pool,
            tc.tile_pool(name="est", bufs=16) as est_pool,
            tc.tile_pool(name="qkt", bufs=3) as qkt_pool,
            tc.tile_pool(name="sgp", bufs=1) as sgp_pool,
            tc.tile_pool(name="stg", bufs=2) as stg_pool,
            tc.tile_pool(name="s2p", bufs=2) as s2_pool,
            tc.tile_pool(name="invr", bufs=4) as invr_pool,
            tc.tile_pool(name="outsb", bufs=2) as outsb_pool,
            tc.tile_pool(name="ps", bufs=1, space="PSUM") as ps,
        ):
            def _ps_tile(shape, tag):
                return ps.tile(
                    shape, F32, tag=tag, bufs=2 if tag in ("bc", "sc") else 1,
                    name=f"ps_{tag}",
                )

            XT = const.tile([128, KT, N], FP16)
            WQKV = const.tile([128, KT, 3 * C], FP16)
            WPROJ = const.tile([128, KT, C], FP16)
            xt_r = xt_ext.rearrange("(t p) n -> p t n", p=128)
            wqkv_r = wqkv_ext.rearrange("(t p) n -> p t n", p=128)
            # the first v-proj accumulation group needs ALL six k-tiles of x
            # and Wv, so issue each as ONE large DMA (two queues run in
            # parallel) instead of 12 small ones — the sync sequencer takes
            # ~0.6us per dma_start and serial issue was costing ~9us of
            # startup before the first matmul
            nc.sync.dma_start(out=XT[:], in_=xt_r[:])
            nc.sync.dma_start(
                out=WQKV[:, :, 2 * C : 3 * C], in_=wqkv_r[:, :, 2 * C : 3 * C]
            )
            nc.sync.dma_start(out=WQKV[:, :, 0 : 2 * C], in_=wqkv_r[:, :, 0 : 2 * C])
            nc.sync.dma_start(out=WPROJ[:], in_=wproj_ext.rearrange("(t p) n -> p t n", p=128))
            if has_bias:
                BQKV = const.tile([1, 3 * C], FP16)
                BPROJ = const.tile([1, C], FP16)
                ONESROW = const.tile([1, N], FP16)
                nc.sync.dma_start(out=BQKV[:], in_=bqkv_ext[:])
                nc.sync.dma_start(out=BPROJ[:], in_=bproj_ext[:])
                nc.vector.memset(ONESROW[:], 1.0)

            ONES1x64 = const.tile([1, 64], FP16)
            nc.vector.memset(ONES1x64[:], 1.0)

            # v in [n, col] layout with a 1.0 column after each head's 64
            # columns: with v_aug as the matmul's stationary operand, output
            # partition 64 accumulates the softmax denominators for free.
            VN65 = const.tile([128, NT, H * E], FP16)
            for h in range(H):
                nc.vector.memset(VN65[:, :, h * E + D : h * E + E], 1.0)
            ATTNT = const.tile([128, KT, N], FP16)
            # softmax denominators parked at partition 64 (same partition the
            # attn@v matmuls write them to — engines cannot shift partitions)
            SGP = sgp_pool.tile([65, 2 * N], F32, tag="sgp")

            # ---- V projection: v[n, vcol] = xT.T @ Wv (+ b_v) ----
            _vtags = ["sc", "ot", "bc", "bc"]
            for nt in range(NT):
                for vs in range(2):
                    pv = _ps_tile([128, 384], _vtags[(nt * 2 + vs) % 4])
                    dst = pv[:, 0:384]
                    for kt in range(KT):
                        nc.tensor.matmul(
                            dst,
                            XT[:, kt, nt * 128 : (nt + 1) * 128],
                            WQKV[:, kt, 2 * C + vs * 384 : 2 * C + (vs + 1) * 384],
                            start=(kt == 0),
                            stop=(kt == KT - 1 and not has_bias),
                        )
                    if has_bias:
                        nc.tensor.matmul(
                            dst,
                            ONESROW[0:1, nt * 128 : (nt + 1) * 128],
                            BQKV[0:1, 2 * C + vs * 384 : 2 * C + (vs + 1) * 384],
                            start=False,
                            stop=True,
                        )
                    nc.vector.tensor_copy(
                        VN65[:, nt, vs * 6 * E : (vs + 1) * 6 * E].rearrange(
                            "p (h e) -> p h e", e=E
                        )[:, :, 0:D],
                        dst.rearrange("p (h d) -> p h d", d=D),
                    )

            # ---- head-pair loop, software-pipelined one pair deep:
            # pair hp:   scores -> exp -> est     (ACT-bound phase)
            # pair hp-1: v_aug.T @ est            (dense PE work, fills gaps)
            # pair hp+1: qT/kT projection chunks  (always-ready PE filler)
            # pair hp-2: deferred normalize broadcast (hides sums-DMA latency)

            qkt_state = {}
            qkt_tiles = {}

            def qkt_half(pair, mt):
                # 24 qkT matmuls spread 4-per-mt over mts 0..5, so the final
                # cast lands two mts before the next pair's scores need it.
                # chunk c = (q ns0, q ns1, k ns0, k ns1); 6 matmuls per chunk.
                if mt >= 6:
                    return
                if mt == 0:
                    qkt_tiles[pair] = qkt_pool.tile(
                        [128, 2 * N], FP16, tag="qkt", name=f"qkt_{pair}"
                    )
                for j in range(4 * mt, 4 * mt + 4):
                    c, kt = j // 6, j % 6
                    ct = pair if c < 2 else HP + pair
                    col0 = ct * 128
                    ns = c % 2
                    if kt == 0:
                        qkt_state[pair] = _ps_tile([128, 512], "bc")
                    pqc = qkt_state[pair]
                    nc.tensor.matmul(
                        pqc[:],
                        WQKV[:, kt, col0 : col0 + 128],
                        XT[:, kt, ns * 512 : (ns + 1) * 512],
                        start=(kt == 0),
                        stop=(kt == KT - 1 and not has_bias),
                    )
                    if kt == KT - 1:
                        if has_bias:
                            nc.tensor.matmul(
                                pqc[:],
                                BQKV[0:1, col0 : col0 + 128],
                                ONESROW[0:1, ns * 512 : (ns + 1) * 512],
                                start=False,
                                stop=True,
                            )
                        nc.vector.tensor_copy(
                            qkt_tiles[pair][:, c * 512 : (c + 1) * 512], pqc[:]
                        )

            for mt in range(8):
                qkt_half(0, mt)

            def do_norm(pend):
                # broadcast 1/sums rows to [128, 512] (head0 -> partitions
                # 0:63, head1 -> 64:127) and normalize ATTNT in place
                ph, IR0, IR1 = pend
                for ns in range(2):
                    nsl = slice(ns * 512, (ns + 1) * 512)
                    BCt = _ps_tile([128, 512], "bc")
                    nc.tensor.matmul(
                        BCt[0:64, :], ONES1x64[:], IR0[0:1, nsl],
                        start=True, stop=True,
                    )
                    nc.tensor.matmul(
                        BCt[64:128, :], ONES1x64[:], IR1[0:1, nsl],
                        start=True, stop=True,
                        tile_position=(0, 64),
                    )
                    nc.vector.tensor_mul(
                        ATTNT[:, ph, nsl], ATTNT[:, ph, nsl], BCt[:]
                    )

            norm_pending = None
            prev = None  # (hp, [EST per mt])
            OUTT = None
            for hp in range(HP + 1):
                if hp < HP:
                    h0, h1 = 2 * hp, 2 * hp + 1
                cur = []
                if prev is not None:
                    ph, pest = prev
                for mt in range(8):
                    # deferred normalize of pair hp-2: its 1/sums rows came
                    # back from the DMA shuffle during iterations 0-1
                    if norm_pending is not None and mt == 2:
                        do_norm(norm_pending)
                        norm_pending = None

                    # next pair's qkT half-chunk: elastic PE filler
                    if hp + 1 < HP:
                        qkt_half(hp + 1, mt)

                    # phase 1 of the current pair (the est = exp * epos DVE
                    # multiply is deferred to the end of the iteration: its
                    # consumer is a whole pair-loop away, and issuing it last
                    # keeps the DVE free for the phase-2 flush copies that
                    # gate the next attn@v accumulation)
                    EPP = ESTP = None
                    if hp < HP:
                        # issue the epos loads from the (otherwise idle)
                        # gpsimd sequencer: the sync sequencer spends ~0.6us
                        # per dma_start and was falling behind at pair
                        # boundaries where the sums-shuffle DMAs burst
                        EPP = epp_pool.tile([128, 2 * N], FP16, tag="epp")
                        nc.gpsimd.dma_start(out=EPP[:, 0:N], in_=epos_ext[h0, mt * 128 : (mt + 1) * 128, :])
                        nc.gpsimd.dma_start(out=EPP[:, N : 2 * N], in_=epos_ext[h1, mt * 128 : (mt + 1) * 128, :])

                        # scores + exp per HEAD in separate half-size psum
                        # tiles (tag ring of 2): next iteration's scores for a
                        # head only wait on THAT head's exp, which finishes
                        # ~1us earlier than a combined-pair exp — the combined
                        # version had zero slack and stalled the PE ~360ns
                        # every iteration
                        QKTh = qkt_tiles[hp]
                        ESTP = est_pool.tile([128, 2 * N], FP16, tag="est")
                        for hh in range(2):
                            row = slice(64 * hh, 64 * hh + 64)
                            SCP = _ps_tile([128, N], "sc")
                            for ns in range(2):
                                nsl = slice(ns * 512, (ns + 1) * 512)
                                nc.tensor.matmul(
                                    SCP[:, nsl],
                                    QKTh[row, N + mt * 128 : N + (mt + 1) * 128],
                                    QKTh[row, nsl],
                                    start=True, stop=True,
                                )
                            nc.scalar.activation(
                                ESTP[:, hh * N : (hh + 1) * N], SCP[:], EXP,
                                scale=SCALE,
                            )
                        cur.append(ESTP)

                    # phase 2 of the previous pair (issued after the scores so
                    # an OUTT slot-reuse wait never blocks them): head0 of the
                    # pair sweeps est m-tiles 0..7 during mts 0-3, head1
                    # during mts 4-7.  outT_h[0:64] = attn@v rows, outT_h[64]
                    # = softmax denominators (from v_aug's ones column).
                    if prev is not None:
                        h_loc, sub = divmod(mt, 4)
                        hg = 2 * ph + h_loc
                        if sub == 0:
                            OUTT = _ps_tile([65, 1024], "ot")
                        for m2 in (2 * sub, 2 * sub + 1):
                            for ns in range(2):
                                nsl = slice(ns * 512, (ns + 1) * 512)
                                nc.tensor.matmul(
                                    OUTT[:, nsl],
                                    VN65[:, m2, hg * E : hg * E + E],
                                    pest[m2][:, h_loc * N + ns * 512 : h_loc * N + (ns + 1) * 512],
                                    start=(m2 == 0), stop=(m2 == 7),
                                )
                        if sub == 3:
                            # flush this head on DVE (ACT is saturated by the
                            # exp chain; GPSIMD has no PSUM port): rows 0:63
                            # -> ATTNT (head0 directly; head1 staged and
                            # repartitioned to 64:128 by an SBUF->SBUF DMA),
                            # denominator row 64 -> SGP (same partition).
                            nc.vector.tensor_copy(
                                SGP[64:65, h_loc * N : (h_loc + 1) * N],
                                OUTT[64:65, :],
                            )
                            if h_loc == 0:
                                nc.vector.tensor_copy(ATTNT[0:64, ph, :], OUTT[0:64, :])
                            else:
                                STG1 = stg_pool.tile([64, N], FP16, tag="stg")
                                nc.vector.tensor_copy(STG1[:], OUTT[0:64, :])
                                nc.sync.dma_start(
                                    out=ATTNT[64:128, ph, :], in_=STG1[:]
                                )
                                # reshape both denominator rows across 64
                                # lanes, invert, and send 1/sums back as
                                # [1, N] fp16 rows for the broadcast matmuls
                                S2 = s2_pool.tile([64, 32], F32, tag="s2")
                                for i in range(2):
                                    nc.sync.dma_start(
                                        out=S2[:, 16 * i : 16 * (i + 1)],
                                        in_=SGP[64:65, i * N : (i + 1) * N].rearrange(
                                            "o (p f) -> o p f", p=64
                                        ),
                                    )
                                RI = s2_pool.tile([64, 32], F32, tag="ri")
                                nc.vector.reciprocal(RI[:], S2[:])
                                RI16 = s2_pool.tile([64, 32], FP16, tag="ri16")
                                nc.vector.tensor_copy(RI16[:], RI[:])
                                IR0 = invr_pool.tile([1, N], FP16, tag="invr")
                                IR1 = invr_pool.tile([1, N], FP16, tag="invr")
                                for i, ir in enumerate((IR0, IR1)):
                                    nc.sync.dma_start(
                                        out=ir[0:1, :].rearrange(
                                            "o (p f) -> o p f", p=64
                                        ),
                                        in_=RI16[:, 16 * i : 16 * (i + 1)],
                                    )
                                norm_pending = (ph, IR0, IR1)

                    # deferred phase-1 tail: est = exp(scores) * epos
                    if ESTP is not None:
                        nc.vector.tensor_mul(ESTP[:], ESTP[:], EPP[:])

                if hp < HP:
                    prev = (hp, cur)

            # tail: normalize the last pair (the only place the sums-DMA
            # latency is exposed), then the output projection
            if norm_pending is not None:
                do_norm(norm_pending)
                norm_pending = None

            # ---- output projection: out[n, c'] = attnT.T @ Wproj (+ b) ----
            for nt in range(NT):
                OF = outsb_pool.tile([128, C], F32, tag="of")
                for cs in range(2):
                    po = _ps_tile([128, 384], _vtags[(nt * 2 + cs) % 4])
                    dst = po[:, 0:384]
                    for ct in range(KT):
                        nc.tensor.matmul(
                            dst,
                            ATTNT[:, ct, nt * 128 : (nt + 1) * 128],
                            WPROJ[:, ct, cs * 384 : (cs + 1) * 384],
                            start=(ct == 0),
                            stop=(ct == KT - 1 and not has_bias),
                        )
                    if has_bias:
                        nc.tensor.matmul(
                            dst,
                            ONESROW[0:1, nt * 128 : (nt + 1) * 128],
                            BPROJ[0:1, cs * 384 : (cs + 1) * 384],
                            start=False,
                            stop=True,
                        )
                    nc.vector.tensor_copy(OF[:, cs * 384 : (cs + 1) * 384], dst)
                nc.sync.dma_start(out=out_ext[nt * 128 : (nt + 1) * 128, :], in_=OF[:])

    _split_excess_waits(nc)
    return nc


_BUILT = {}


def _get_nc(has_bias):
    if has_bias not in _BUILT:
        _BUILT[has_bias] = build(has_bias)
    return _BUILT[has_bias]


def prepare_inputs(x, pos_embedding, W_qkv, b_qkv, W_proj, b_proj):
    B = x.shape[0]
    has_bias = bool(np.any(b_qkv)) or bool(np.any(b_proj))
    wqkv16 = np.ascontiguousarray(W_qkv).astype(np.float16)
    wproj16 = np.ascontiguousarray(W_proj).astype(np.float16)
    epos16 = np.exp(
        pos_embedding[0].transpose(0, 2, 1).astype(np.float32) * SCALE
    ).astype(np.float16)
    epos16 = np.ascontiguousarray(epos16)
    in_maps = []
    for b in range(B):
        m = {
            "xt": np.ascontiguousarray(x[b].T).astype(np.float16),
            "wqkv": wqkv16,
            "wproj": wproj16,
            "epos": epos16,
        }
        if has_bias:
            m["bqkv"] = b_qkv.reshape(1, -1).astype(np.float16)
            m["bproj"] = b_proj.reshape(1, -1).astype(np.float16)
        in_maps.append(m)
    return has_bias, in_maps


def kernel(x, pos_embedding, W_qkv, b_qkv, W_proj, b_proj):
    from concourse.bass_utils import run_bass_kernel_spmd

    x = np.asarray(x)
    pos_embedding = np.asarray(pos_embedding)
    W_qkv = np.asarray(W_qkv)
    b_qkv = np.asarray(b_qkv)
    W_proj = np.asarray(W_proj)
    b_proj = np.asarray(b_proj)

    has_bias, in_maps = prepare_inputs(x, pos_embedding, W_qkv, b_qkv, W_proj, b_proj)
    nc = _get_nc(has_bias)
    res = run_bass_kernel_spmd(nc, in_maps, list(range(N_CORES)), trace=False)
    out = np.stack([res.results[i]["out"] for i in range(N_CORES)], axis=0)
    return out.astype(np.float32)


# revision 34
# speedup vs baseline: 1.2144x; 1.0187x over previous
"""Multi-head self-attention with additive position bias, data-parallel across
8 TRN2 NeuronCores (one batch element per core).

Per core (batch b), everything is computed in a transposed layout so that no
on-device transposes are needed:
  - host supplies xT = x[b].T (fp16) and epos[h] = exp(pos[h].T / sqrt(D)) (fp16)
  - qT/kT    = W_{q,k}.T @ xT                    [cols, N]   (PE, fp16)
  - v        = xT.T @ W_v                        [N, cols]   (PE, fp16), stored
               with a literal 1.0 column after each head's 64 columns
  - scoresT  = kT_h(m-tile).T @ qT_h             [m, n]      (PE, head-pairs
               packed into row groups 0-63 / 64-127 of the systolic array)
  - estT     = exp(scoresT/sqrt(D)) * eposT      (ACT exp + DVE mul; the
               additive bias becomes a multiplicative factor after exp)
  - outT_h   = v_aug,h.T @ estT : [65, n] accumulated over m-tiles.  The
               stationary operand is the 65-column v_aug (cheap LDWEIGHTS,
               fully hidden under the 512-element streams), and partition 64
               receives the softmax denominators for free — no separate
               ones-vector matmuls.
  - normalization: denominators are reshaped across lanes via two tiny DMAs,
    inverted on DVE, DMA'd back to [1, N] rows, broadcast with one-row
    matmuls, and multiplied into ATTNT.  The broadcast is deferred into the
    NEXT pair's loop so the small-DMA latency chain never stalls the PE.
  - out      = attnT.T @ W_proj                  [N, C] fp32
"""

import numpy as np

N_CORES = 8
N = 1024
C = 768
H = 12
D = 64
E = D + 1  # 65: v columns + ones column per head
HP = H // 2  # head pairs
SCALE = 0.125  # 1/sqrt(D)

# ---------------------------------------------------------------------------
# walrus in this toolchain rejects instructions carrying more than one sync
# wait ("Too many sync wait commands").  Tile's semaphore pass can attach
# several (esp. the kernel-tail drain).  Spread surplus waits across InstNoOp
# instructions inserted immediately before the oversubscribed instruction in
# the same basic block / engine stream — semantically identical, since the
# engine sequencer performs the waits in stream order.
# ---------------------------------------------------------------------------


def _apply_tile_patch():
    from concourse import mybir
    from concourse.tile import TileContext
    from concourse.vector_clock import ScopedClock

    def _patched_drain_and_barrier(self, tick_clock, wait_clock):
        nc = self.nc
        drain_inst = nc.sync.drain()
        wait_clock.add_sem_waits(
            drain_inst.ins, ScopedClock({None: tick_clock.global_clock})
        )
        nc.all_engine_barrier()
        assert self.sems is not None
        popped = nc._tile_sem_poison_stack.pop()
        assert popped is self._sem_poison
        nc.clear_and_free_semaphores(list(self.sems.allocated().values()))
        nc.all_engine_barrier()

    TileContext._drain_and_barrier = _patched_drain_and_barrier


def _split_excess_waits(nc, max_waits=1):
    from concourse import mybir

    n_split = 0
    for f in nc.m.functions:
        for blk in f.blocks:
            insts = blk.instructions
            new_list = []
            changed = False
            for inst in insts:
                si = inst.sync_info
                waits = list(si.on_wait) if (si is not None and si.on_wait) else []
                eff_max = max_waits
                if len(waits) > eff_max:
                    step = max(eff_max, 1)
                    extra = waits if eff_max == 0 else waits[: len(waits) - eff_max]
                    keep = [] if eff_max == 0 else waits[len(waits) - eff_max :]
                    for i in range(0, len(extra), step):
                        nop = mybir.InstNoOp(
                            name=nc.get_next_instruction_name(),
                            engine=inst.engine,
                            ins=[],
                            outs=[],
                            sync_info=mybir.SyncInfo(
                                on_wait=extra[i : i + step], on_update=[]
                            ),
                        )
                        nc.register_instruction(nop, overwrite=True)
                        new_list.append(nop)
                        n_split += 1
                    inst.sync_info = mybir.SyncInfo(
                        on_wait=keep,
                        on_update=list(si.on_update) if si.on_update else [],
                    )
                    changed = True
                new_list.append(inst)
            if changed:
                blk.instructions = new_list
    return n_split


def build(has_bias):
    import concourse.bass as bass
    import concourse.mybir as mybir
    from concourse.tile import TileContext

    _apply_tile_patch()

    FP16 = mybir.dt.float16
    F32 = mybir.dt.float32
    EXP = mybir.ActivationFunctionType.Exp

    nc = bass.Bass()
    xt_ext = nc.declare_dram_parameter("xt", [C, N], FP16, isOutput=False)
    wqkv_ext = nc.declare_dram_parameter("wqkv", [C, 3 * C], FP16, isOutput=False)
    wproj_ext = nc.declare_dram_parameter("wproj", [C, C], FP16, isOutput=False)
    epos_ext = nc.declare_dram_parameter("epos", [H, N, N], FP16, isOutput=False)
    if has_bias:
        bqkv_ext = nc.declare_dram_parameter("bqkv", [1, 3 * C], FP16, isOutput=False)
        bproj_ext = nc.declare_dram_parameter("bproj", [1, C], FP16, isOutput=False)
    out_ext = nc.declare_dram_parameter("out", [N, C], F32, isOutput=True)

    KT = C // 128  # 6 contraction tiles
    NT = N // 128  # 8 n-tiles / m-tiles

    with TileContext(nc) as tc:
        with (
            tc.tile_pool(name="const", bufs=1) as const,
            tc.tile_pool(name="epp", bufs=4) as epp_pool,
            tc.tile_pool(name="est", bufs=16) as est_pool,
            tc.tile_pool(name="qkt", bufs=3) as qkt_pool,
            tc.tile_pool(name="sgp", bufs=1) as sgp_pool,
            tc.tile_pool(name="stg", bufs=2) as stg_pool,
            tc.tile_pool(name="s2p", bufs=2) as s2_pool,
            tc.tile_pool(name="invr", bufs=4) as invr_pool,
            tc.tile_pool(name="outsb", bufs=2) as outsb_pool,
            tc.tile_pool(name="ps", bufs=1, space="PSUM") as ps,
        ):
            def _ps_tile(shape, tag):
                return ps.tile(
                    shape, F32, tag=tag, bufs=2 if tag in ("bc", "sc") else 1,
                    name=f"ps_{tag}",
                )

            XT = const.tile([128, KT, N], FP16)
            WQKV = const.tile([128, KT, 3 * C], FP16)
            WPROJ = const.tile([128, KT, C], FP16)
            xt_r = xt_ext.rearrange("(t p) n -> p t n", p=128)
            wqkv_r = wqkv_ext.rearrange("(t p) n -> p t n", p=128)
            # Startup is HBM-bandwidth-bound: the first v-proj group needs
            # ALL of x and Wv, so those two loads go first as single large
            # DMAs with nothing else competing.  Wq/Wk follow (needed ~15us
            # later by the qkT prelude), and Wproj — needed only by the final
            # projection — trails everything.
            for k2 in range(3):
                nc.sync.dma_start(
                    out=XT[:, 2 * k2 : 2 * k2 + 2, :], in_=xt_r[:, 2 * k2 : 2 * k2 + 2, :]
                )
            for vh in range(2):
                nc.scalar.dma_start(
                    out=WQKV[:, 3 * vh : 3 * vh + 3, 2 * C : 3 * C],
                    in_=wqkv_r[:, 3 * vh : 3 * vh + 3, 2 * C : 3 * C],
                )
            nc.sync.dma_start(out=WQKV[:, :, 0 : 2 * C], in_=wqkv_r[:, :, 0 : 2 * C])
            if has_bias:
                BQKV = const.tile([1, 3 * C], FP16)
                BPROJ = const.tile([1, C], FP16)
                ONESROW = const.tile([1, N], FP16)
                nc.sync.dma_start(out=BQKV[:], in_=bqkv_ext[:])
                nc.sync.dma_start(out=BPROJ[:], in_=bproj_ext[:])
                nc.vector.memset(ONESROW[:], 1.0)

            ONES1x64 = const.tile([1, 64], FP16)
            nc.vector.memset(ONES1x64[:], 1.0)

            # v in [n, col] layout with a 1.0 column after each head's 64
            # columns: with v_aug as the matmul's stationary operand, output
            # partition 64 accumulates the softmax denominators for free.
            VN65 = const.tile([128, NT, H * E], FP16)
            for h in range(H):
                nc.vector.memset(VN65[:, :, h * E + D : h * E + E], 1.0)
            ATTNT = const.tile([128, KT, N], FP16)
            # softmax denominators parked at partition 64 (same partition the
            # attn@v matmuls write them to — engines cannot shift partitions)
            SGP = sgp_pool.tile([65, 2 * N], F32, tag="sgp")

            # ---- V projection: v[n, vcol] = xT.T @ Wv (+ b_v) ----
            _vtags = ["sc", "ot", "bc", "bc"]
            for nt in range(NT):
                for vs in range(2):
                    pv = _ps_tile([128, 384], _vtags[(nt * 2 + vs) % 4])
                    dst = pv[:, 0:384]
                    for kt in range(KT):
                        nc.tensor.matmul(
                            dst,
                            XT[:, kt, nt * 128 : (nt + 1) * 128],
                            WQKV[:, kt, 2 * C + vs * 384 : 2 * C + (vs + 1) * 384],
                            start=(kt == 0),
                            stop=(kt == KT - 1 and not has_bias),
                        )
                    if has_bias:
                        nc.tensor.matmul(
                            dst,
                            ONESROW[0:1, nt * 128 : (nt + 1) * 128],
                            BQKV[0:1, 2 * C + vs * 384 : 2 * C + (vs + 1) * 384],
                            start=False,
                            stop=True,
                        )
                    nc.vector.tensor_copy(
                        VN65[:, nt, vs * 6 * E : (vs + 1) * 6 * E].rearrange(
                            "p (h e) -> p h e", e=E
                        )[:, :, 0:D],
                        dst.rearrange("p (h d) -> p h d", d=D),
                    )

            # ---- head-pair loop, software-pipelined one pair deep:
            # pair hp:   scores -> exp -> est     (ACT-bound phase)
            # pair hp-1: v_aug.T @ est            (dense PE work, fills gaps)
            # pair hp+1: qT/kT projection chunks  (always-ready PE filler)
            # pair hp-2: deferred normalize broadcast (hides sums-DMA latency)

            qkt_state = {}
            qkt_tiles = {}

            def qkt_half(pair, mt):
                # 24 qkT matmuls spread 4-per-mt over mts 0..5, so the final
                # cast lands two mts before the next pair's scores need it.
                # chunk c = (q ns0, q ns1, k ns0, k ns1); 6 matmuls per chunk.
                if mt >= 6:
                    return
                if mt == 0:
                    qkt_tiles[pair] = qkt_pool.tile(
                        [128, 2 * N], FP16, tag="qkt", name=f"qkt_{pair}"
                    )
                for j in range(4 * mt, 4 * mt + 4):
                    c, kt = j // 6, j % 6
                    ct = pair if c < 2 else HP + pair
                    col0 = ct * 128
                    ns = c % 2
                    if kt == 0:
                        qkt_state[pair] = _ps_tile([128, 512], "bc")
                    pqc = qkt_state[pair]
                    nc.tensor.matmul(
                        pqc[:],
                        WQKV[:, kt, col0 : col0 + 128],
                        XT[:, kt, ns * 512 : (ns + 1) * 512],
                        start=(kt == 0),
                        stop=(kt == KT - 1 and not has_bias),
                    )
                    if kt == KT - 1:
                        if has_bias:
                            nc.tensor.matmul(
                                pqc[:],
                                BQKV[0:1, col0 : col0 + 128],
                                ONESROW[0:1, ns * 512 : (ns + 1) * 512],
                                start=False,
                                stop=True,
                            )
                        nc.vector.tensor_copy(
                            qkt_tiles[pair][:, c * 512 : (c + 1) * 512], pqc[:]
                        )

            for mt in range(8):
                qkt_half(0, mt)

            def do_norm(pend):
                # broadcast 1/sums rows to [128, 512] (head0 -> partitions
                # 0:63, head1 -> 64:127) and normalize ATTNT in place
                ph, IR0, IR1 = pend
                for ns in range(2):
                    nsl = slice(ns * 512, (ns + 1) * 512)
                    BCt = _ps_tile([128, 512], "bc")
                    nc.tensor.matmul(
                        BCt[0:64, :], ONES1x64[:], IR0[0:1, nsl],
                        start=True, stop=True,
                    )
                    nc.tensor.matmul(
                        BCt[64:128, :], ONES1x64[:], IR1[0:1, nsl],
                        start=True, stop=True,
                        tile_position=(0, 64),
                    )
                    nc.vector.tensor_mul(
                        ATTNT[:, ph, nsl], ATTNT[:, ph, nsl], BCt[:]
                    )

            norm_pending = None
            prev = None  # (hp, [EST per mt])
            OUTT = None
            for hp in range(HP + 1):
                if hp < HP:
                    h0, h1 = 2 * hp, 2 * hp + 1
                cur = []
                if prev is not None:
                    ph, pest = prev
                for mt in range(8):
                    # deferred normalize of pair hp-2: its 1/sums rows came
                    # back from the DMA shuffle during iterations 0-1
                    if norm_pending is not None and mt == 6:
                        do_norm(norm_pending)
                        norm_pending = None

                    # next pair's qkT half-chunk: elastic PE filler
                    if hp + 1 < HP:
                        qkt_half(hp + 1, mt)

                    # phase 1 of the current pair (the est = exp * epos DVE
                    # multiply is deferred to the end of the iteration: its
                    # consumer is a whole pair-loop away, and issuing it last
                    # keeps the DVE free for the phase-2 flush copies that
                    # gate the next attn@v accumulation)
                    EPP = ESTP = None
                    if hp < HP:
                        # epos loads stay on the sync sequencer so they queue
                        # BEHIND the critical startup input loads; the bursty
                        # pair-boundary shuffle DMAs go to gpsimd instead
                        EPP = epp_pool.tile([128, 2 * N], FP16, tag="epp")
                        nc.sync.dma_start(out=EPP[:, 0:N], in_=epos_ext[h0, mt * 128 : (mt + 1) * 128, :])
                        nc.sync.dma_start(out=EPP[:, N : 2 * N], in_=epos_ext[h1, mt * 128 : (mt + 1) * 128, :])
                        if hp == 0 and mt == 1:
                            # Wproj is only needed by the final projection;
                            # issuing it here keeps it off the critical
                            # startup bandwidth
                            nc.sync.dma_start(
                                out=WPROJ[:],
                                in_=wproj_ext.rearrange("(t p) n -> p t n", p=128),
                            )

                        # scores + exp per HEAD in separate half-size psum
                        # tiles (tag ring of 2): next iteration's scores for a
                        # head only wait on THAT head's exp, which finishes
                        # ~1us earlier than a combined-pair exp — the combined
                        # version had zero slack and stalled the PE ~360ns
                        # every iteration
                        QKTh = qkt_tiles[hp]
                        ESTP = est_pool.tile([128, 2 * N], FP16, tag="est")
                        for hh in range(2):
                            row = slice(64 * hh, 64 * hh + 64)
                            SCP = _ps_tile([128, N], "sc")
                            for ns in range(2):
                                nsl = slice(ns * 512, (ns + 1) * 512)
                                nc.tensor.matmul(
                                    SCP[:, nsl],
                                    QKTh[row, N + mt * 128 : N + (mt + 1) * 128],
                                    QKTh[row, nsl],
                                    start=True, stop=True,
                                )
                            nc.scalar.activation(
                                ESTP[:, hh * N : (hh + 1) * N], SCP[:], EXP,
                                scale=SCALE,
                            )
                        cur.append(ESTP)

                    # phase 2 of the previous pair (issued after the scores so
                    # an OUTT slot-reuse wait never blocks them): head0 of the
                    # pair sweeps est m-tiles 0..7 during mts 0-3, head1
                    # during mts 4-7.  outT_h[0:64] = attn@v rows, outT_h[64]
                    # = softmax denominators (from v_aug's ones column).
                    if prev is not None:
                        h_loc, sub = divmod(mt, 4)
                        hg = 2 * ph + h_loc
                        if sub == 0:
                            OUTT = _ps_tile([65, 1024], "ot")
                        for m2 in (2 * sub, 2 * sub + 1):
                            for ns in range(2):
                                nsl = slice(ns * 512, (ns + 1) * 512)
                                nc.tensor.matmul(
                                    OUTT[:, nsl],
                                    VN65[:, m2, hg * E : hg * E + E],
                                    pest[m2][:, h_loc * N + ns * 512 : h_loc * N + (ns + 1) * 512],
                                    start=(m2 == 0), stop=(m2 == 7),
                                )
                        if sub == 3:
                            # flush this head on DVE (ACT is saturated by the
                            # exp chain; GPSIMD has no PSUM port): rows 0:63
                            # -> ATTNT (head0 directly; head1 staged and
                            # repartitioned to 64:128 by an SBUF->SBUF DMA),
                            # denominator row 64 -> SGP (same partition).
                            nc.vector.tensor_copy(
                                SGP[64:65, h_loc * N : (h_loc + 1) * N],
                                OUTT[64:65, :],
                            )
                            if h_loc == 0:
                                nc.vector.tensor_copy(ATTNT[0:64, ph, :], OUTT[0:64, :])
                            else:
                                STG1 = stg_pool.tile([64, N], FP16, tag="stg")
                                nc.vector.tensor_copy(STG1[:], OUTT[0:64, :])
                                nc.sync.dma_start(
                                    out=ATTNT[64:128, ph, :], in_=STG1[:]
                                )
                                # reshape both denominator rows across 64
                                # lanes, invert, and send 1/sums back as
                                # [1, N] fp16 rows for the broadcast matmuls
                                S2 = s2_pool.tile([64, 32], F32, tag="s2")
                                for i in range(2):
                                    nc.sync.dma_start(
                                        out=S2[:, 16 * i : 16 * (i + 1)],
                                        in_=SGP[64:65, i * N : (i + 1) * N].rearrange(
                                            "o (p f) -> o p f", p=64
                                        ),
                                    )
                                RI = s2_pool.tile([64, 32], F32, tag="ri")
                                nc.vector.reciprocal(RI[:], S2[:])
                                RI16 = s2_pool.tile([64, 32], FP16, tag="ri16")
                                nc.vector.tensor_copy(RI16[:], RI[:])
                                IR0 = invr_pool.tile([1, N], FP16, tag="invr")
                                IR1 = invr_pool.tile([1, N], FP16, tag="invr")
                                for i, ir in enumerate((IR0, IR1)):
                                    nc.sync.dma_start(
                                        out=ir[0:1, :].rearrange(
                                            "o (p f) -> o p f", p=64
                                        ),
                                        in_=RI16[:, 16 * i : 16 * (i + 1)],
                                    )
                                norm_pending = (ph, IR0, IR1)

                    # deferred phase-1 tail: est = exp(scores) * epos
                    if ESTP is not None:
                        nc.vector.tensor_mul(ESTP[:], ESTP[:], EPP[:])

                if hp < HP:
                    prev = (hp, cur)

            # tail: normalize the last pair one n-half at a time, running the
            # projection n-tiles of each half as soon as that half is
            # normalized — the second broadcast hides under the first four
            # projection tiles
            def do_norm_half(pend, ns):
                ph, IR0, IR1 = pend
                nsl = slice(ns * 512, (ns + 1) * 512)
                BCt = _ps_tile([128, 512], "bc")
                nc.tensor.matmul(
                    BCt[0:64, :], ONES1x64[:], IR0[0:1, nsl],
                    start=True, stop=True,
                )
                nc.tensor.matmul(
                    BCt[64:128, :], ONES1x64[:], IR1[0:1, nsl],
                    start=True, stop=True,
                    tile_position=(0, 64),
                )
                nc.vector.tensor_mul(
                    ATTNT[:, ph, nsl], ATTNT[:, ph, nsl], BCt[:]
                )

            if norm_pending is not None:
                do_norm(norm_pending)
                norm_pending = None

            # ---- output projection: out[n, c'] = attnT.T @ Wproj (+ b) ----
            for nt in range(NT):
                OF = outsb_pool.tile([128, C], F32, tag="of")
                for cs in range(2):
                    po = _ps_tile([128, 384], _vtags[(nt * 2 + cs) % 4])
                    dst = po[:, 0:384]
                    for ct in range(KT):
                        nc.tensor.matmul(
                            dst,
                            ATTNT[:, ct, nt * 128 : (nt + 1) * 128],
                            WPROJ[:, ct, cs * 384 : (cs + 1) * 384],
                            start=(ct == 0),
                            stop=(ct == KT - 1 and not has_bias),
                        )
                    if has_bias:
                        nc.tensor.matmul(
                            dst,
                            ONESROW[0:1, nt * 128 : (nt + 1) * 128],
                            BPROJ[0:1, cs * 384 : (cs + 1) * 384],
                            start=False,
                            stop=True,
                        )
                    nc.vector.tensor_copy(OF[:, cs * 384 : (cs + 1) * 384], dst)
                # spread the tail's eight output stores across three
                # sequencers: serial issue on sync alone costs ~0.64us each
                _oeng = (nc.sync, nc.gpsimd, nc.scalar)[nt % 3]
                _oeng.dma_start(out=out_ext[nt * 128 : (nt + 1) * 128, :], in_=OF[:])

    _split_excess_waits(nc)
    return nc


_BUILT = {}


def _get_nc(has_bias):
    if has_bias not in _BUILT:
        _BUILT[has_bias] = build(has_bias)
    return _BUILT[has_bias]


def prepare_inputs(x, pos_embedding, W_qkv, b_qkv, W_proj, b_proj):
    B = x.shape[0]
    has_bias = bool(np.any(b_qkv)) or bool(np.any(b_proj))
    wqkv16 = np.ascontiguousarray(W_qkv).astype(np.float16)
    wproj16 = np.ascontiguousarray(W_proj).astype(np.float16)
    epos16 = np.exp(
        pos_embedding[0].transpose(0, 2, 1).astype(np.float32) * SCALE
    ).astype(np.float16)
    epos16 = np.ascontiguousarray(epos16)
    in_maps = []
    for b in range(B):
        m = {
            "xt": np.ascontiguousarray(x[b].T).astype(np.float16),
            "wqkv": wqkv16,
            "wproj": wproj16,
            "epos": epos16,
        }
        if has_bias:
            m["bqkv"] = b_qkv.reshape(1, -1).astype(np.float16)
            m["bproj"] = b_proj.reshape(1, -1).astype(np.float16)
        in_maps.append(m)
    return has_bias, in_maps


def kernel(x, pos_embedding, W_qkv, b_qkv, W_proj, b_proj):
    from concourse.bass_utils import run_bass_kernel_spmd

    x = np.asarray(x)
    pos_embedding = np.asarray(pos_embedding)
    W_qkv = np.asarray(W_qkv)
    b_qkv = np.asarray(b_qkv)
    W_proj = np.asarray(W_proj)
    b_proj = np.asarray(b_proj)

    has_bias, in_maps = prepare_inputs(x, pos_embedding, W_qkv, b_qkv, W_proj, b_proj)
    nc = _get_nc(has_bias)
    res = run_bass_kernel_spmd(nc, in_maps, list(range(N_CORES)), trace=False)
    out = np.stack([res.results[i]["out"] for i in range(N_CORES)], axis=0)
    return out.astype(np.float32)
